# revision 6
# baseline (speedup 1.0000x reference)
import sys, os
sys.path.insert(0, "/opt/trn_rl_repo")
import numpy as np

NCORES = 8
B = 128           # block size
NBT = 32          # matrix block-rows (4096/128)
MATW = 512        # per-core matrix columns (256 src + 256 tgt)
WSL = 513         # slab width: 512 matrix cols + 1 border (y) col
NS_ITERS = 3
KSTEPS = int(os.environ.get('KSTEPS', '32'))
NOCC = os.environ.get('NOCC', '0') == '1'
CONST = 0.5 * 2048 * float(np.log(2 * np.pi))

def f_off(k):
    """global block index -> column offset in the gathered row [128, 4096]."""
    if k < 16:
        return 512 * (k // 2) + 128 * (k % 2)
    return 512 * ((k - 16) // 2) + 256 + 128 * ((k - 16) % 2)


_NC_CACHE = None

def build():
    global _NC_CACHE
    if _NC_CACHE is not None:
        return _NC_CACHE
    import concourse.bass as bass
    import concourse.bacc as bacc
    import concourse.tile as tile
    import concourse.mybir as mybir
    dt = mybir.dt
    A_ = mybir.AluOpType
    AF = mybir.ActivationFunctionType
    SHARED = "Local" if NOCC else "Shared"

    nc = bacc.Bacc("TRN2", target_bir_lowering=False, debug=False, num_devices=NCORES)
    def din(name, shape):
        return nc.dram_tensor(name, shape, dt.float32, kind="ExternalInput").ap()
    xsT = din("xsT", [256, 256]);  xtT = din("xtT", [256, 256])
    bsT = din("bsT", [256, 512]);  btT = din("btT", [256, 512])
    WfT = din("WfT", [256, 8])
    WaS = din("WaS", [128, 8]);    WaT = din("WaT", [128, 8])
    WcS = din("WcS", [8, 1]);      WcT = din("WcT", [8, 1])
    bS = din("bS", [1, 1]);        bT = din("bT", [1, 1])
    noiseS = din("noiseS", [1, 1]); noiseT = din("noiseT", [1, 1])
    Kb_in = din("Kb", [512, 512])
    khad = din("khad", [4096, 512])
    ybcol = din("ybcol", [4224, 1])
    yrow = din("yrow", [1, WSL])
    ones_row = din("ones_row", [1, 128])
    Imask_in = din("Imask", [128, 128])
    onescol = din("onescol", [128, 1])
    nsel0 = din("nsel0", [128, 16]); nsel1 = din("nsel1", [128, 16])
    nsel2 = din("nsel2", [128, 16]); nsel3 = din("nsel3", [128, 16])
    loss_out = nc.dram_tensor("loss", [1, 1], dt.float32, kind="ExternalOutput").ap()

    with tile.TileContext(nc) as tc:
        with tc.tile_pool(name="pers", bufs=1) as P, \
             tc.tile_pool(name="strm", bufs=2) as S, \
             tc.tile_pool(name="pw", bufs=2, space="PSUM") as PW, \
             tc.tile_pool(name="psq", bufs=2, space="PSUM") as PQ, \
             tc.tile_pool(name="pt", bufs=1, space="PSUM") as PT, \
             tc.tile_pool(name="dram", bufs=2, space="DRAM") as DR:

            # ---------- constants into SBUF ----------
            Imask = P.tile([128, 128], dt.float32, tag="Imask", name="Imask")
            nc.sync.dma_start(out=Imask[:, :], in_=Imask_in[:, :])
            onesr = P.tile([1, 128], dt.float32, tag="onesr", name="onesr")
            nc.sync.dma_start(out=onesr[:, :], in_=ones_row[:, :])
            onesc = P.tile([128, 1], dt.float32, tag="onesc", name="onesc")
            nc.sync.dma_start(out=onesc[:, :], in_=onescol[:, :])

            # ---------- A slab (+ single-row border) + pivot-row bf16 slabs ----------
            Arow = [P.tile([128, WSL], dt.float32, tag=f"A{i}", name=f"A{i}") for i in range(NBT)]
            Arow.append(P.tile([1, WSL], dt.float32, tag="A32", name="A32"))
            rowb = [P.tile([128, WSL], dt.bfloat16, tag=f"rowb{j}", name=f"rowb{j}") for j in range(4)]

            # ---------- stage 1 scratch ----------
            st1 = tc.tile_pool(name="st1", bufs=1)
            P1 = st1.__enter__()
            wlocb = [P1.tile([128, 512], dt.bfloat16, tag=f"wlocb{q}", name=f"wlocb{q}") for q in range(4)]
            Kbb = [P1.tile([128, 512], dt.bfloat16, tag=f"Kbb{j}", name=f"Kbb{j}") for j in range(4)]
            wKT = [P1.tile([128, 4096], dt.bfloat16, tag=f"wKT{q}", name=f"wKT{q}") for q in range(4)]

            # Kb loads early so the DMA overlaps the w build
            for j in range(4):
                kbt = S.tile([128, 512], dt.float32, tag="kbt", name="kbt")
                nc.sync.dma_start(out=kbt[:, :], in_=Kb_in[128 * j:128 * j + 128, :])
                nc.vector.tensor_copy(out=Kbb[j][:, :], in_=kbt[:, :])

            # ---------- stage 1: w build (writes wlocb directly) ----------
            def build_w(xT, bT_, Wa_in, Wc, bias, m_off):
                xt = [S.tile([128, 256], dt.float32, tag="xt", name="xt") for _ in range(2)]
                nc.sync.dma_start(out=xt[0][:, :], in_=xT[0:128, :])
                nc.sync.dma_start(out=xt[1][:, :], in_=xT[128:256, :])
                bt_ = [S.tile([128, 512], dt.float32, tag="bt", name="bt") for _ in range(2)]
                nc.sync.dma_start(out=bt_[0][:, :], in_=bT_[0:128, :])
                nc.sync.dma_start(out=bt_[1][:, :], in_=bT_[128:256, :])
                wft = [S.tile([128, 8], dt.float32, tag="wft", name="wft") for _ in range(2)]
                nc.sync.dma_start(out=wft[0][:, :], in_=WfT[0:128, :])
                nc.sync.dma_start(out=wft[1][:, :], in_=WfT[128:256, :])
                wa = S.tile([128, 8], dt.float32, tag="wa", name="wa")
                nc.sync.dma_start(out=wa[:, :], in_=Wa_in[:, :])
                wc = S.tile([8, 1], dt.float32, tag="wc", name="wc")
                nc.sync.dma_start(out=wc[:, :], in_=Wc[:, :])
                bia = S.tile([1, 1], dt.float32, tag="bia", name="bia")
                nc.sync.dma_start(out=bia[:, :], in_=bias[:, :])

                # gT = Wf @ base^T : [8, 512]
                pg = PW.tile([8, 512], dt.float32, tag="pw", name="pw")
                for kc in range(2):
                    nc.tensor.matmul(pg[:, :], wft[kc][:, :], bt_[kc][:, :],
                                     start=(kc == 0), stop=(kc == 1))
                gT = S.tile([8, 512], dt.float32, tag="gT", name="gT")
                nc.vector.tensor_copy(out=gT[:, :], in_=pg[:, :])
                # crow = Wc^T @ gT + bias : [1, 512]
                pc = PT.tile([1, 512], dt.float32, tag="pt", name="pt")
                nc.tensor.matmul(pc[:, :], wc[:, :], gT[:, :], start=True, stop=True)
                crow = S.tile([1, 512], dt.float32, tag="crow", name="crow")
                nc.scalar.activation(out=crow[:, :], in_=pc[:, :], func=AF.Identity,
                                     bias=bia[:, :], scale=1.0)
                # bcast crow to 128 partitions
                pb = PW.tile([128, 512], dt.float32, tag="pw", name="pw")
                nc.tensor.matmul(pb[:, :], onesr[:, :], crow[:, :], start=True, stop=True)

                for r in range(2):
                    pf = PQ.tile([128, 8], dt.float32, tag="psq", name="psq")
                    for kc in range(2):
                        nc.tensor.matmul(pf[:, :], xt[kc][:, 128 * r:128 * r + 128],
                                         wft[kc][:, :], start=(kc == 0), stop=(kc == 1))
                    f_r = S.tile([128, 8], dt.float32, tag="f_r", name="f_r")
                    nc.vector.tensor_copy(out=f_r[:, :], in_=pf[:, :])
                    scr8 = S.tile([128, 8], dt.float32, tag="scr8", name="scr8")
                    a_r = S.tile([128, 1], dt.float32, tag="a_r", name="a_r")
                    nc.vector.tensor_tensor(out=scr8[:, :], in0=f_r[:, :], in1=wa[:, :], op=A_.mult)
                    nc.vector.tensor_reduce(out=a_r[:, :], in_=scr8[:, :],
                                            axis=mybir.AxisListType.X, op=A_.add)
                    raw = S.tile([128, 512], dt.float32, tag="raw", name="raw")
                    nc.vector.tensor_scalar(out=raw[:, :], in0=pb[:, :], scalar1=a_r[:, :],
                                            scalar2=None, op0=A_.add)
                    mn = S.tile([128, 512], dt.float32, tag="mn", name="mn")
                    nc.vector.tensor_scalar(out=mn[:, :], in0=raw[:, :], scalar1=0.0,
                                            scalar2=None, op0=A_.min)
                    ex = S.tile([128, 512], dt.float32, tag="ex", name="ex")
                    nc.scalar.activation(out=ex[:, :], in_=mn[:, :], func=AF.Exp)
                    mx = S.tile([128, 512], dt.float32, tag="mx", name="mx")
                    nc.vector.tensor_scalar(out=mx[:, :], in0=raw[:, :], scalar1=0.0,
                                            scalar2=None, op0=A_.max)
                    w_r = S.tile([128, 512], dt.float32, tag="w_r", name="w_r")
                    nc.vector.scalar_tensor_tensor(out=w_r[:, :], in0=ex[:, :], scalar=-1.0,
                                                   in1=mx[:, :], op0=A_.add, op1=A_.add)
                    sm = S.tile([128, 1], dt.float32, tag="sm", name="sm")
                    nc.vector.tensor_reduce(out=sm[:, :], in_=w_r[:, :],
                                            axis=mybir.AxisListType.X, op=A_.add)
                    rec = S.tile([128, 1], dt.float32, tag="rec", name="rec")
                    nc.vector.reciprocal(out=rec[:, :], in_=sm[:, :])
                    nc.vector.tensor_scalar(out=w_r[:, :], in0=w_r[:, :], scalar1=rec[:, :],
                                            scalar2=512.0, op0=A_.mult, op1=A_.mult)
                    # transpose 128x128 chunks straight into bf16 wlocb
                    for q in range(4):
                        ptr = PQ.tile([128, 128], dt.float32, tag="psq", name="psq")
                        nc.tensor.matmul(ptr[:, :], w_r[:, 128 * q:128 * q + 128], Imask[:, :], start=True, stop=True)
                        nc.vector.tensor_copy(
                            out=wlocb[q][:, m_off + 128 * r: m_off + 128 * r + 128],
                            in_=ptr[:, :])

            build_w(xsT, bsT, WaS, WcS, bS, 0)
            build_w(xtT, btT, WaT, WcT, bT, 256)

            # ---------- local (w Kb)^T chunks, single bf16 gather ----------
            gin = DR.tile([128, 2048], dt.bfloat16, tag="gin", name="gin")
            gout = DR.tile([1024, 2048], dt.bfloat16, tag="gout", name="gout",
                           addr_space=SHARED)
            for qo in range(4):
                pg2 = PW.tile([128, 512], dt.float32, tag="pw", name="pw")
                for jt in range(4):
                    nc.tensor.matmul(pg2[:, :], Kbb[jt][:, 128 * qo:128 * qo + 128],
                                     wlocb[jt][:, :], start=(jt == 0), stop=(jt == 3))
                wkc = S.tile([128, 512], dt.bfloat16, tag="wkc", name="wkc")
                nc.vector.tensor_copy(out=wkc[:, :], in_=pg2[:, :])
                nc.gpsimd.dma_start(out=gin[:, 512 * qo:512 * qo + 512], in_=wkc[:, :])
            if NOCC:
                for c in range(NCORES):
                    nc.gpsimd.dma_start(out=gout[128 * c:128 * c + 128, :], in_=gin[:, :])
            else:
                nc.gpsimd.collective_compute("AllGather", A_.bypass,
                                             replica_groups=[list(range(NCORES))],
                                             ins=[gin[:, :].opt()], outs=[gout[:, :].opt()])
            # scatter gathered (w Kb)^T into wKT[q] ([ws; wt] global column order)
            for c in range(NCORES):
                for q in range(4):
                    nc.sync.dma_start(out=wKT[q][:, 256 * c:256 * c + 256],
                                      in_=gout[128 * c:128 * c + 128, 512 * q:512 * q + 256])
                    nc.scalar.dma_start(out=wKT[q][:, 2048 + 256 * c:2048 + 256 * c + 256],
                                        in_=gout[128 * c:128 * c + 128, 512 * q + 256:512 * q + 512])

            # ---------- noise prep ----------
            def noise_bcast(nin, tagp):
                cl = S.tile([1, 1], dt.float32, tag=f"cl{tagp}", name=f"cl{tagp}")
                nin_sb = S.tile([1, 1], dt.float32, tag=f"ni{tagp}", name=f"ni{tagp}")
                nc.sync.dma_start(out=nin_sb[:, :], in_=nin[:, :])
                nc.vector.tensor_scalar(out=cl[:, :], in0=nin_sb[:, :], scalar1=1e-5,
                                        scalar2=1.0, op0=A_.max, op1=A_.min)
                pn = PT.tile([128, 1], dt.float32, tag="pt", name="pt")
                nc.tensor.matmul(pn[:, :], onesr[:, :], cl[:, :], start=True, stop=True)
                nb = P.tile([128, 1], dt.float32, tag=f"nb{tagp}", name=f"nb{tagp}")
                nc.vector.tensor_copy(out=nb[:, :], in_=pn[:, :])
                return nb
            nbS = noise_bcast(noiseS, "S")
            nbT = noise_bcast(noiseT, "T")
            nsc = []
            for idx, (nsin, nb) in enumerate([(nsel0, nbS), (nsel1, nbS), (nsel2, nbT), (nsel3, nbT)]):
                t = S.tile([128, 16], dt.float32, tag=f"nselt{idx}", name=f"nselt{idx}")
                nc.sync.dma_start(out=t[:, :], in_=nsin[:, :])
                sc = P.tile([128, 16], dt.float32, tag=f"nsc{idx}", name=f"nsc{idx}")
                nc.vector.tensor_scalar(out=sc[:, :], in0=t[:, :], scalar1=nb[:, :],
                                        scalar2=None, op0=A_.mult)
                nsc.append(sc)

            # ---------- A build + pivot-gather prologue ----------
            nc.sync.dma_start(out=Arow[NBT][0:1, :], in_=yrow[0:1, 0:WSL])

            def build_A_row(i):
                pm = PW.tile([128, 512], dt.float32, tag="pw", name="pw")
                for qt in range(4):
                    nc.tensor.matmul(pm[:, :], wKT[qt][:, 128 * i:128 * i + 128],
                                     wlocb[qt][:, :], start=(qt == 0), stop=(qt == 3))
                kh = S.tile([128, 512], dt.float32, tag="kh", name="kh", bufs=3)
                nc.sync.dma_start(out=kh[:, :], in_=khad[128 * i:128 * i + 128, :])
                nc.vector.scalar_tensor_tensor(out=Arow[i][:, 0:512], in0=pm[:, :], scalar=1.0,
                                               in1=kh[:, :], op0=A_.mult, op1=A_.mult)
                if i < 16:
                    nc.vector.scalar_tensor_tensor(
                        out=Arow[i][:, 0:128], in0=Imask[:, :], scalar=nsc[0][:, i:i + 1],
                        in1=Arow[i][:, 0:128], op0=A_.mult, op1=A_.add)
                    nc.vector.scalar_tensor_tensor(
                        out=Arow[i][:, 128:256], in0=Imask[:, :], scalar=nsc[1][:, i:i + 1],
                        in1=Arow[i][:, 128:256], op0=A_.mult, op1=A_.add)
                else:
                    nc.vector.scalar_tensor_tensor(
                        out=Arow[i][:, 256:384], in0=Imask[:, :], scalar=nsc[2][:, i - 16:i - 15],
                        in1=Arow[i][:, 256:384], op0=A_.mult, op1=A_.add)
                    nc.vector.scalar_tensor_tensor(
                        out=Arow[i][:, 384:512], in0=Imask[:, :], scalar=nsc[3][:, i - 16:i - 15],
                        in1=Arow[i][:, 384:512], op0=A_.mult, op1=A_.add)
                nc.scalar.dma_start(out=Arow[i][:, 512:513], in_=ybcol[128 * i:128 * i + 128, :])

            def prologue(k):
                """cast pivot row k, push to DRAM, start its AllGather"""
                kk = k % 4
                nc.vector.tensor_copy(out=rowb[kk][:, :], in_=Arow[k][:, :])
                ci = DR.tile([128, 512], dt.bfloat16, tag="cinb", name="cinb")
                nc.gpsimd.dma_start(out=ci[:, :], in_=rowb[kk][:, 0:512])
                co = DR.tile([1024, 512], dt.bfloat16, tag="coutb", name="coutb",
                             bufs=3, addr_space=SHARED)
                if NOCC:
                    for c in range(NCORES):
                        nc.gpsimd.dma_start(out=co[128 * c:128 * c + 128, :], in_=ci[:, :])
                else:
                    nc.gpsimd.collective_compute("AllGather", A_.bypass,
                                                 replica_groups=[list(range(NCORES))],
                                                 ins=[ci[:, :].opt()], outs=[co[:, :].opt()])
                return co

            for i in range(4):
                build_A_row(i)
            pend = {}
            if KSTEPS > 0:
                pend[0] = prologue(0)
            for i in range(4, NBT):
                build_A_row(i)

            st1.__exit__(None, None, None)

            # ---------- stage 2: elimination ----------
            st2 = tc.tile_pool(name="st2", bufs=1)
            P2 = st2.__enter__()
            grow = [P2.tile([128, 4096], dt.bfloat16, tag=f"grow{j}", name=f"grow{j}") for j in range(4)]
            Vb = [P2.tile([128, WSL], dt.bfloat16, tag=f"Vb{j}", name=f"Vb{j}") for j in range(4)]
            ld_acc = [P2.tile([128, 1], dt.float32, tag=f"ld{j}", name=f"ld{j}") for j in range(2)]
            nc.vector.memset(ld_acc[0][:, :], 0.0)
            ld_cur = 0
            cmid = P2.tile([1, 1], dt.float32, tag="cmid", name="cmid")
            cend = P2.tile([1, 1], dt.float32, tag="cend", name="cend")
            nc.vector.memset(cmid[:, :], 0.0)
            nc.vector.memset(cend[:, :], 0.0)

            def upd_row_single(r, kk):
                pu = PW.tile([128, WSL], dt.float32, tag="pw", name="pw")
                nc.tensor.matmul(pu[:, 0:512], grow[kk][:, f_off(r):f_off(r) + 128],
                                 Vb[kk][:, 0:512], start=True, stop=True)
                nc.tensor.matmul(pu[:, 512:513], grow[kk][:, f_off(r):f_off(r) + 128],
                                 Vb[kk][:, 512:513], start=True, stop=True)
                nc.vector.scalar_tensor_tensor(out=Arow[r][:, 0:512], in0=Arow[r][:, 0:512],
                                               scalar=1.0, in1=pu[:, 0:512],
                                               op0=A_.mult, op1=A_.subtract)
                nc.vector.scalar_tensor_tensor(out=Arow[r][:, 512:513], in0=Arow[r][:, 512:513],
                                               scalar=1.0, in1=pu[:, 512:513],
                                               op0=A_.mult, op1=A_.subtract)

            def upd_row_batch(r):
                if r < NBT:
                    pu = PW.tile([128, WSL], dt.float32, tag="pw", name="pw")
                    for k2 in range(4):
                        nc.tensor.matmul(pu[:, 0:512], grow[k2][:, f_off(r):f_off(r) + 128],
                                         Vb[k2][:, 0:512], start=(k2 == 0), stop=(k2 == 3))
                    for k2 in range(4):
                        nc.tensor.matmul(pu[:, 512:513], grow[k2][:, f_off(r):f_off(r) + 128],
                                         Vb[k2][:, 512:513], start=(k2 == 0), stop=(k2 == 3))
                    nc.vector.scalar_tensor_tensor(out=Arow[r][:, 0:512],
                                                   in0=Arow[r][:, 0:512], scalar=1.0,
                                                   in1=pu[:, 0:512], op0=A_.mult, op1=A_.subtract)
                    nc.vector.scalar_tensor_tensor(out=Arow[r][:, 512:513],
                                                   in0=Arow[r][:, 512:513], scalar=1.0,
                                                   in1=pu[:, 512:513], op0=A_.mult, op1=A_.subtract)
                else:
                    pu = PT.tile([1, WSL], dt.float32, tag="pt", name="pt")
                    for k2 in range(4):
                        nc.tensor.matmul(pu[0:1, 0:512], rowb[k2][:, 512:513],
                                         Vb[k2][:, 0:512], start=(k2 == 0), stop=(k2 == 3))
                    for k2 in range(4):
                        nc.tensor.matmul(pu[0:1, 512:513], rowb[k2][:, 512:513],
                                         Vb[k2][:, 512:513], start=(k2 == 0), stop=(k2 == 3))
                    nc.vector.scalar_tensor_tensor(out=Arow[NBT][0:1, 0:512],
                                                   in0=Arow[NBT][0:1, 0:512], scalar=1.0,
                                                   in1=pu[0:1, 0:512], op0=A_.mult, op1=A_.subtract)
                    nc.vector.scalar_tensor_tensor(out=Arow[NBT][0:1, 512:513],
                                                   in0=Arow[NBT][0:1, 512:513], scalar=1.0,
                                                   in1=pu[0:1, 512:513], op0=A_.mult, op1=A_.subtract)

            for k in range(KSTEPS):
                kk = k % 4
                sup = k // 4
                co = pend.pop(k)
                # pull gathered row into SBUF; diag chunk first, immediate-update
                # chunks next, rest after
                c_dk = f_off(k) // 512
                order = [c_dk]
                for r in range(k + 1, 4 * sup + 4):
                    c = f_off(r) // 512
                    if c not in order:
                        order.append(c)
                for c in range(NCORES):
                    if c not in order:
                        order.append(c)
                for idx, c in enumerate(order):
                    eng = nc.sync if idx % 2 == 0 else nc.scalar
                    eng.dma_start(out=grow[kk][:, 512 * c:512 * c + 512],
                                  in_=co[128 * c:128 * c + 128, :])
                # pivot diag inverse via Newton-Schulz
                Dk = S.tile([128, 128], dt.float32, tag="Dk", name="Dk")
                nc.vector.tensor_copy(out=Dk[:, :], in_=grow[kk][:, f_off(k):f_off(k) + 128])
                scr = S.tile([128, 128], dt.float32, tag="scrD", name="scrD")
                dg = S.tile([128, 1], dt.float32, tag="dg", name="dg")
                nc.vector.tensor_tensor(out=scr[:, :], in0=Dk[:, :], in1=Imask[:, :], op=A_.mult)
                nc.vector.tensor_reduce(out=dg[:, :], in_=scr[:, :],
                                        axis=mybir.AxisListType.X, op=A_.add)
                rcp = S.tile([128, 1], dt.float32, tag="rcp", name="rcp")
                nc.vector.reciprocal(out=rcp[:, :], in_=dg[:, :])
                X = S.tile([128, 128], dt.float32, tag="Xns", name="Xns")
                nc.vector.tensor_scalar(out=X[:, :], in0=Imask[:, :], scalar1=rcp[:, :],
                                        scalar2=None, op0=A_.mult)
                for it in range(NS_ITERS):
                    pT = PQ.tile([128, 128], dt.float32, tag="psq", name="psq")
                    nc.tensor.matmul(pT[:, :], Dk[:, :], X[:, :], start=True, stop=True)
                    Z = S.tile([128, 128], dt.float32, tag="Zns", name="Zns")
                    nc.vector.scalar_tensor_tensor(out=Z[:, :], in0=Imask[:, :], scalar=2.0,
                                                   in1=pT[:, :], op0=A_.mult, op1=A_.subtract)
                    pX = PQ.tile([128, 128], dt.float32, tag="psq", name="psq")
                    nc.tensor.matmul(pX[:, :], X[:, :], Z[:, :], start=True, stop=True)
                    X = S.tile([128, 128], dt.float32, tag="Xns", name="Xns")
                    nc.vector.tensor_copy(out=X[:, :], in_=pX[:, :])
                INVb = S.tile([128, 128], dt.bfloat16, tag="INVb", name="INVb")
                nc.vector.tensor_copy(out=INVb[:, :], in_=X[:, :])
                # V = INV @ row_k (local slab width)
                pv = PW.tile([128, WSL], dt.float32, tag="pw", name="pw")
                nc.tensor.matmul(pv[:, 0:512], INVb[:, :], rowb[kk][:, 0:512], start=True, stop=True)
                nc.tensor.matmul(pv[:, 512:513], INVb[:, :], rowb[kk][:, 512:513], start=True, stop=True)
                nc.vector.tensor_copy(out=Vb[kk][:, 0:512], in_=pv[:, 0:512])
                nc.vector.tensor_copy(out=Vb[kk][:, 512:513], in_=pv[:, 512:513])

                # updates; next pivot row first, then its gather, then the rest
                if kk < 3:
                    upd_row_single(k + 1, kk)
                    pend[k + 1] = prologue(k + 1)
                    for r in range(k + 2, 4 * sup + 4):
                        upd_row_single(r, kk)
                else:
                    rows = list(range(4 * sup + 4, NBT + 1))
                    if k + 1 < KSTEPS:
                        upd_row_batch(rows[0])
                        pend[k + 1] = prologue(k + 1)
                        for r in rows[1:]:
                            upd_row_batch(r)
                    else:
                        for r in rows:
                            upd_row_batch(r)
                    if k == 15:
                        nc.vector.tensor_copy(out=cmid[:, :], in_=Arow[NBT][0:1, 512:513])
                    if k == KSTEPS - 1:
                        nc.vector.tensor_copy(out=cend[:, :], in_=Arow[NBT][0:1, 512:513])

                # logdet pieces (off the gather critical path)
                if k >= 16:
                    lg = S.tile([128, 1], dt.float32, tag="lg", name="lg")
                    nc.scalar.activation(out=lg[:, :], in_=dg[:, :], func=AF.Ln)
                    sq = S.tile([128, 1], dt.float32, tag="sq", name="sq")
                    nc.scalar.activation(out=sq[:, :], in_=rcp[:, :], func=AF.Sqrt)
                    T1 = S.tile([128, 128], dt.float32, tag="T1", name="T1")
                    nc.vector.tensor_scalar(out=T1[:, :], in0=Dk[:, :], scalar1=sq[:, :],
                                            scalar2=None, op0=A_.mult)
                    psr = PT.tile([1, 128], dt.float32, tag="pt", name="pt")
                    nc.tensor.matmul(psr[:, :], sq[:, :], Imask[:, :], start=True, stop=True)
                    sqr = S.tile([1, 128], dt.float32, tag="sqr", name="sqr")
                    nc.vector.tensor_copy(out=sqr[:, :], in_=psr[:, :])
                    pbc = PQ.tile([128, 128], dt.float32, tag="psq", name="psq")
                    nc.tensor.matmul(pbc[:, :], onesr[:, :], sqr[:, :], start=True, stop=True)
                    E1 = S.tile([128, 128], dt.float32, tag="E1", name="E1")
                    nc.vector.scalar_tensor_tensor(out=E1[:, :], in0=T1[:, :], scalar=1.0,
                                                   in1=pbc[:, :], op0=A_.mult, op1=A_.mult)
                    E = S.tile([128, 128], dt.float32, tag="Emat", name="Emat")
                    nc.vector.scalar_tensor_tensor(out=E[:, :], in0=E1[:, :], scalar=1.0,
                                                   in1=Imask[:, :], op0=A_.mult, op1=A_.subtract)
                    pows = [E]
                    # E2, E3, E4, E5
                    for (la, lb) in [(0, 0), (1, 0), (1, 1), (3, 0)]:
                        pp = PQ.tile([128, 128], dt.float32, tag="psq", name="psq")
                        nc.tensor.matmul(pp[:, :], pows[la][:, :], pows[lb][:, :],
                                         start=True, stop=True)
                        Ei = S.tile([128, 128], dt.float32, tag=f"E{len(pows) + 1}", name=f"E{len(pows) + 1}")
                        nc.vector.tensor_copy(out=Ei[:, :], in_=pp[:, :])
                        pows.append(Ei)
                    E2, E3, E4, E5 = pows[1], pows[2], pows[3], pows[4]
                    pairs = [(E, Imask, 1), (E, E, 2), (E2, E, 3), (E2, E2, 4), (E3, E2, 5),
                             (E3, E3, 6), (E4, E3, 7), (E4, E4, 8), (E5, E4, 9), (E5, E5, 10)]
                    ser = None
                    for (Pa, Pb, order_) in pairs:
                        scr2 = S.tile([128, 128], dt.float32, tag="scr2", name="scr2")
                        tr = S.tile([128, 1], dt.float32, tag=f"tr{order_}", name=f"tr{order_}")
                        nc.vector.tensor_tensor(out=scr2[:, :], in0=Pa[:, :], in1=Pb[:, :], op=A_.mult)
                        nc.vector.tensor_reduce(out=tr[:, :], in_=scr2[:, :],
                                                axis=mybir.AxisListType.X, op=A_.add)
                        coef = ((-1.0) ** (order_ + 1)) / order_
                        if ser is None:
                            ser = S.tile([128, 1], dt.float32, tag="ser", name="ser")
                            nc.vector.tensor_scalar(out=ser[:, :], in0=tr[:, :], scalar1=coef,
                                                    scalar2=None, op0=A_.mult)
                        else:
                            ser2 = S.tile([128, 1], dt.float32, tag="ser", name="ser")
                            nc.vector.scalar_tensor_tensor(out=ser2[:, :], in0=tr[:, :],
                                                           scalar=coef, in1=ser[:, :],
                                                           op0=A_.mult, op1=A_.add)
                            ser = ser2
                    tot = S.tile([128, 1], dt.float32, tag="totld", name="totld")
                    nc.vector.scalar_tensor_tensor(out=tot[:, :], in0=lg[:, :], scalar=1.0,
                                                   in1=ser[:, :], op0=A_.mult, op1=A_.add)
                    nxt = 1 - ld_cur
                    nc.vector.scalar_tensor_tensor(out=ld_acc[nxt][:, :], in0=tot[:, :],
                                                   scalar=1.0, in1=ld_acc[ld_cur][:, :],
                                                   op0=A_.mult, op1=A_.add)
                    ld_cur = nxt

            # ---------- finale ----------
            pld = PT.tile([1, 1], dt.float32, tag="pt", name="pt")
            nc.tensor.matmul(pld[:, :], ld_acc[ld_cur][:, :], onesc[:, :], start=True, stop=True)
            ldsum = S.tile([1, 1], dt.float32, tag="ldsum", name="ldsum")
            nc.vector.tensor_copy(out=ldsum[:, :], in_=pld[:, :])
            qd = S.tile([1, 1], dt.float32, tag="qd", name="qd")
            nc.vector.scalar_tensor_tensor(out=qd[:, :], in0=cmid[:, :], scalar=1.0,
                                           in1=cend[:, :], op0=A_.mult, op1=A_.subtract)
            b1 = S.tile([1, 1], dt.float32, tag="b1", name="b1")
            nc.vector.tensor_scalar(out=b1[:, :], in0=qd[:, :], scalar1=0.5,
                                    scalar2=CONST, op0=A_.mult, op1=A_.add)
            lossv = S.tile([1, 1], dt.float32, tag="lossv", name="lossv")
            nc.scalar.activation(out=lossv[:, :], in_=ldsum[:, :], func=AF.Identity,
                                 bias=b1[:, :], scale=0.25)
            nc.sync.dma_start(out=loss_out[:, :], in_=lossv[:, :])
            st2.__exit__(None, None, None)

    nc.compile()
    _NC_CACHE = nc
    return nc


LAST_EXEC_NS = None

def kernel(**inputs):
    global LAST_EXEC_NS
    from concourse.bass_utils import run_bass_kernel_spmd
    f32 = np.float32

    def arr(x):
        return np.ascontiguousarray(np.asarray(x, dtype=f32))

    sx, tx = arr(inputs["source_x"]), arr(inputs["target_x"])
    sy, ty = arr(inputs["source_y"]), arr(inputs["target_y"])
    k_ss, k_tt, k_st = arr(inputs["k_ss"]), arr(inputs["k_tt"]), arr(inputs["k_st"])
    Wf, bf = arr(inputs["Wf"]), arr(inputs["bf"])
    Ws, bs = arr(inputs["Ws"]), arr(inputs["bs"])
    Wt, bt = arr(inputs["Wt"]), arr(inputs["bt"])
    Kb = arr(inputs["Kb"])
    base_s, base_t = arr(inputs["base_s"]), arr(inputs["base_t"])
    noise_s, noise_t = arr(inputs["noise_s_opt"]), arr(inputs["noise_t_opt"])

    assert np.all(bf == 0.0), "kernel assumes bf == 0"
    ybcol = np.concatenate([sy[:, 0], ty[:, 0], np.zeros(128, f32)]).reshape(4224, 1).astype(f32)
    ones_row = np.ones((1, 128), f32)
    Imask = np.eye(128, dtype=f32)
    onescol = np.ones((128, 1), f32)
    WfT = np.ascontiguousarray(Wf.T)

    in_maps = []
    for c in range(NCORES):
        s0 = 256 * c
        nsel = [np.zeros((128, 16), f32) for _ in range(4)]
        nsel[0][:, 2 * c] = 1.0
        nsel[1][:, 2 * c + 1] = 1.0
        nsel[2][:, 2 * c] = 1.0
        nsel[3][:, 2 * c + 1] = 1.0
        khad_c = np.empty((4096, 512), f32)
        khad_c[0:2048, 0:256] = k_ss[:, s0:s0 + 256]
        khad_c[0:2048, 256:512] = k_st[:, s0:s0 + 256]
        khad_c[2048:4096, 0:256] = k_st[s0:s0 + 256, :].T
        khad_c[2048:4096, 256:512] = k_tt[:, s0:s0 + 256]
        yrow = np.zeros((1, WSL), f32)
        yrow[0, 0:256] = sy[s0:s0 + 256, 0]
        yrow[0, 256:512] = ty[s0:s0 + 256, 0]
        in_maps.append(dict(
            xsT=np.ascontiguousarray(sx[s0:s0 + 256, :].T),
            xtT=np.ascontiguousarray(tx[s0:s0 + 256, :].T),
            bsT=np.ascontiguousarray(base_s.T),
            btT=np.ascontiguousarray(base_t.T),
            WfT=WfT,
            WaS=np.tile(Ws[0:1, 0:8], (128, 1)).astype(f32),
            WaT=np.tile(Wt[0:1, 0:8], (128, 1)).astype(f32),
            WcS=np.ascontiguousarray(Ws[0, 8:16].reshape(8, 1)),
            WcT=np.ascontiguousarray(Wt[0, 8:16].reshape(8, 1)),
            bS=bs.reshape(1, 1), bT=bt.reshape(1, 1),
            noiseS=noise_s.reshape(1, 1), noiseT=noise_t.reshape(1, 1),
            Kb=Kb, khad=khad_c, ybcol=ybcol, yrow=yrow,
            ones_row=ones_row, Imask=Imask, onescol=onescol,
            nsel0=nsel[0], nsel1=nsel[1], nsel2=nsel[2], nsel3=nsel[3],
        ))

    nc = build()
    trace = bool(int(os.environ.get("KERNEL_TRACE", "0")))
    loss = None
    if os.environ.get("KERNEL_FORCE_SIM", "0") != "1":
        try:
            kw = {}
            td = os.environ.get("KERNEL_TRACE_DIR")
            if td:
                kw["tmpdir"] = td
            res = run_bass_kernel_spmd(nc, in_maps, core_ids=list(range(NCORES)), trace=trace, **kw)
            LAST_EXEC_NS = res.exec_time_ns
            loss = np.float32(res.results[0]["loss"][0, 0])
        except Exception as e:
            sys.stderr.write("HW path failed (%s); falling back to MultiCoreSim\n" % type(e).__name__)
    if loss is None or not np.isfinite(loss):
        from concourse.bass_interp import MultiCoreSim
        sim = MultiCoreSim(nc, num_cores=NCORES, trace=False,
                           require_finite=False, require_nnan=False)
        for i in range(NCORES):
            for kk, vv in in_maps[i].items():
                sim.cores[i].tensor(kk)[:] = vv
        sim.simulate(check_with_hw=False)
        loss = np.float32(sim.cores[0].mem_tensor("loss")[0, 0])
    return np.asarray(loss, dtype=np.float32).reshape(())


# revision 15
# speedup vs baseline: 1.1741x; 1.1741x over previous
import sys, os
sys.path.insert(0, "/opt/trn_rl_repo")
import numpy as np

NCORES = 8
B = 128           # block size
NBT = 32          # matrix block-rows (4096/128)
MATW = 512        # per-core matrix columns (256 src + 256 tgt)
WSL = 513         # slab width: 512 matrix cols + 1 border (y) col
NS_ITERS = 3
KSTEPS = int(os.environ.get('KSTEPS', '32'))
NOCC = os.environ.get('NOCC', '0') == '1'
CONST = 0.5 * 2048 * float(np.log(2 * np.pi))

def f_off(k):
    """global block index -> column offset in the gathered row [128, 4096]."""
    if k < 16:
        return 512 * (k // 2) + 128 * (k % 2)
    return 512 * ((k - 16) // 2) + 256 + 128 * ((k - 16) % 2)


_NC_CACHE = None

def build():
    global _NC_CACHE
    if _NC_CACHE is not None:
        return _NC_CACHE
    import concourse.bass as bass
    import concourse.bacc as bacc
    import concourse.tile as tile
    import concourse.mybir as mybir
    dt = mybir.dt
    A_ = mybir.AluOpType
    AF = mybir.ActivationFunctionType
    SHARED = "Local" if NOCC else "Shared"

    nc = bacc.Bacc("TRN2", target_bir_lowering=False, debug=False, num_devices=NCORES)
    def din(name, shape):
        return nc.dram_tensor(name, shape, dt.float32, kind="ExternalInput").ap()
    xsT = din("xsT", [256, 256]);  xtT = din("xtT", [256, 256])
    bsT = din("bsT", [256, 512]);  btT = din("btT", [256, 512])
    WfT = din("WfT", [256, 8])
    WaS = din("WaS", [128, 8]);    WaT = din("WaT", [128, 8])
    WcS = din("WcS", [8, 1]);      WcT = din("WcT", [8, 1])
    bS = din("bS", [1, 1]);        bT = din("bT", [1, 1])
    noiseS = din("noiseS", [1, 1]); noiseT = din("noiseT", [1, 1])
    Kb_in = din("Kb", [512, 512])
    khad = din("khad", [4096, 512])
    ybcol = din("ybcol", [4224, 1])
    yrow = din("yrow", [1, WSL])
    ones_row = din("ones_row", [1, 128])
    Imask_in = din("Imask", [128, 128])
    onescol = din("onescol", [128, 1])
    nsel0 = din("nsel0", [128, 16]); nsel1 = din("nsel1", [128, 16])
    nsel2 = din("nsel2", [128, 16]); nsel3 = din("nsel3", [128, 16])
    loss_out = nc.dram_tensor("loss", [1, 1], dt.float32, kind="ExternalOutput").ap()

    with tile.TileContext(nc) as tc:
        with tc.tile_pool(name="pers", bufs=1) as P, \
             tc.tile_pool(name="strm", bufs=2) as S, \
             tc.tile_pool(name="pw", bufs=2, space="PSUM") as PW, \
             tc.tile_pool(name="psq", bufs=2, space="PSUM") as PQ, \
             tc.tile_pool(name="pt", bufs=1, space="PSUM") as PT, \
             tc.tile_pool(name="dram", bufs=2, space="DRAM") as DR:

            # ---------- constants into SBUF ----------
            Imask = P.tile([128, 128], dt.float32, tag="Imask", name="Imask")
            nc.sync.dma_start(out=Imask[:, :], in_=Imask_in[:, :])
            onesr = P.tile([1, 128], dt.float32, tag="onesr", name="onesr")
            nc.sync.dma_start(out=onesr[:, :], in_=ones_row[:, :])
            onesc = P.tile([128, 1], dt.float32, tag="onesc", name="onesc")
            nc.sync.dma_start(out=onesc[:, :], in_=onescol[:, :])

            # ---------- A slab (+ single-row border) + pivot-row bf16 slabs ----------
            Arow = [P.tile([128, WSL], dt.float32, tag=f"A{i}", name=f"A{i}") for i in range(NBT)]
            Arow.append(P.tile([1, WSL], dt.float32, tag="A32", name="A32"))
            rowb = [P.tile([128, WSL], dt.bfloat16, tag=f"rowb{j}", name=f"rowb{j}") for j in range(4)]
            grow = [P.tile([128, 4096], dt.bfloat16, tag=f"grow{j}", name=f"grow{j}") for j in range(4)]

            # ---------- stage 1 scratch ----------
            st1 = tc.tile_pool(name="st1", bufs=1)
            P1 = st1.__enter__()
            wlocb = [P1.tile([128, 512], dt.bfloat16, tag=f"wlocb{q}", name=f"wlocb{q}") for q in range(4)]
            Kbb = [P1.tile([128, 512], dt.bfloat16, tag=f"Kbb{j}", name=f"Kbb{j}") for j in range(4)]
            wKT = [P1.tile([128, 4096], dt.bfloat16, tag=f"wKT{q}", name=f"wKT{q}") for q in range(4)]

            # Kb loads early so the DMA overlaps the w build
            for j in range(4):
                kbt = S.tile([128, 512], dt.float32, tag="kbt", name="kbt")
                nc.sync.dma_start(out=kbt[:, :], in_=Kb_in[128 * j:128 * j + 128, :])
                nc.vector.tensor_copy(out=Kbb[j][:, :], in_=kbt[:, :])

            # ---------- stage 1: w build (writes wlocb directly) ----------
            def build_w(xT, bT_, Wa_in, Wc, bias, m_off):
                xt = [S.tile([128, 256], dt.float32, tag="xt", name="xt") for _ in range(2)]
                nc.sync.dma_start(out=xt[0][:, :], in_=xT[0:128, :])
                nc.sync.dma_start(out=xt[1][:, :], in_=xT[128:256, :])
                bt_ = [S.tile([128, 512], dt.float32, tag="bt", name="bt") for _ in range(2)]
                nc.sync.dma_start(out=bt_[0][:, :], in_=bT_[0:128, :])
                nc.sync.dma_start(out=bt_[1][:, :], in_=bT_[128:256, :])
                wft = [S.tile([128, 8], dt.float32, tag="wft", name="wft") for _ in range(2)]
                nc.sync.dma_start(out=wft[0][:, :], in_=WfT[0:128, :])
                nc.sync.dma_start(out=wft[1][:, :], in_=WfT[128:256, :])
                wa = S.tile([128, 8], dt.float32, tag="wa", name="wa")
                nc.sync.dma_start(out=wa[:, :], in_=Wa_in[:, :])
                wc = S.tile([8, 1], dt.float32, tag="wc", name="wc")
                nc.sync.dma_start(out=wc[:, :], in_=Wc[:, :])
                bia = S.tile([1, 1], dt.float32, tag="bia", name="bia")
                nc.sync.dma_start(out=bia[:, :], in_=bias[:, :])

                # gT = Wf @ base^T : [8, 512]
                pg = PW.tile([8, 512], dt.float32, tag="pw", name="pw")
                for kc in range(2):
                    nc.tensor.matmul(pg[:, :], wft[kc][:, :], bt_[kc][:, :],
                                     start=(kc == 0), stop=(kc == 1))
                gT = S.tile([8, 512], dt.float32, tag="gT", name="gT")
                nc.vector.tensor_copy(out=gT[:, :], in_=pg[:, :])
                # crow = Wc^T @ gT + bias : [1, 512]
                pc = PT.tile([1, 512], dt.float32, tag="pt", name="pt")
                nc.tensor.matmul(pc[:, :], wc[:, :], gT[:, :], start=True, stop=True)
                crow = S.tile([1, 512], dt.float32, tag="crow", name="crow")
                nc.scalar.activation(out=crow[:, :], in_=pc[:, :], func=AF.Identity,
                                     bias=bia[:, :], scale=1.0)
                # bcast crow to 128 partitions
                pb = PW.tile([128, 512], dt.float32, tag="pw", name="pw")
                nc.tensor.matmul(pb[:, :], onesr[:, :], crow[:, :], start=True, stop=True)

                for r in range(2):
                    pf = PQ.tile([128, 8], dt.float32, tag="psq", name="psq")
                    for kc in range(2):
                        nc.tensor.matmul(pf[:, :], xt[kc][:, 128 * r:128 * r + 128],
                                         wft[kc][:, :], start=(kc == 0), stop=(kc == 1))
                    f_r = S.tile([128, 8], dt.float32, tag="f_r", name="f_r")
                    nc.vector.tensor_copy(out=f_r[:, :], in_=pf[:, :])
                    scr8 = S.tile([128, 8], dt.float32, tag="scr8", name="scr8")
                    a_r = S.tile([128, 1], dt.float32, tag="a_r", name="a_r")
                    nc.vector.tensor_tensor(out=scr8[:, :], in0=f_r[:, :], in1=wa[:, :], op=A_.mult)
                    nc.vector.tensor_reduce(out=a_r[:, :], in_=scr8[:, :],
                                            axis=mybir.AxisListType.X, op=A_.add)
                    raw = S.tile([128, 512], dt.float32, tag="raw", name="raw", bufs=1)
                    nc.vector.tensor_scalar(out=raw[:, :], in0=pb[:, :], scalar1=a_r[:, :],
                                            scalar2=None, op0=A_.add)
                    mn = S.tile([128, 512], dt.float32, tag="mn", name="mn", bufs=1)
                    nc.vector.tensor_scalar(out=mn[:, :], in0=raw[:, :], scalar1=0.0,
                                            scalar2=None, op0=A_.min)
                    ex = S.tile([128, 512], dt.float32, tag="ex", name="ex", bufs=1)
                    nc.scalar.activation(out=ex[:, :], in_=mn[:, :], func=AF.Exp)
                    mx = S.tile([128, 512], dt.float32, tag="mx", name="mx", bufs=1)
                    nc.vector.tensor_scalar(out=mx[:, :], in0=raw[:, :], scalar1=0.0,
                                            scalar2=None, op0=A_.max)
                    w_r = S.tile([128, 512], dt.float32, tag="w_r", name="w_r")
                    nc.vector.scalar_tensor_tensor(out=w_r[:, :], in0=ex[:, :], scalar=-1.0,
                                                   in1=mx[:, :], op0=A_.add, op1=A_.add)
                    sm = S.tile([128, 1], dt.float32, tag="sm", name="sm")
                    nc.vector.tensor_reduce(out=sm[:, :], in_=w_r[:, :],
                                            axis=mybir.AxisListType.X, op=A_.add)
                    rec = S.tile([128, 1], dt.float32, tag="rec", name="rec")
                    nc.vector.reciprocal(out=rec[:, :], in_=sm[:, :])
                    nc.vector.tensor_scalar(out=w_r[:, :], in0=w_r[:, :], scalar1=rec[:, :],
                                            scalar2=512.0, op0=A_.mult, op1=A_.mult)
                    # transpose 128x128 chunks straight into bf16 wlocb
                    for q in range(4):
                        ptr = PQ.tile([128, 128], dt.float32, tag="psq", name="psq")
                        nc.tensor.matmul(ptr[:, :], w_r[:, 128 * q:128 * q + 128], Imask[:, :], start=True, stop=True)
                        nc.vector.tensor_copy(
                            out=wlocb[q][:, m_off + 128 * r: m_off + 128 * r + 128],
                            in_=ptr[:, :])

            build_w(xsT, bsT, WaS, WcS, bS, 0)
            build_w(xtT, btT, WaT, WcT, bT, 256)

            # ---------- local (w Kb)^T chunks, single bf16 gather ----------
            gin = DR.tile([128, 2048], dt.bfloat16, tag="gin", name="gin")
            gout = DR.tile([1024, 2048], dt.bfloat16, tag="gout", name="gout",
                           addr_space=SHARED)
            for qo in range(4):
                pg2 = PW.tile([128, 512], dt.float32, tag="pw", name="pw")
                for jt in range(4):
                    nc.tensor.matmul(pg2[:, :], Kbb[jt][:, 128 * qo:128 * qo + 128],
                                     wlocb[jt][:, :], start=(jt == 0), stop=(jt == 3))
                wkc = S.tile([128, 512], dt.bfloat16, tag="wkc", name="wkc")
                nc.vector.tensor_copy(out=wkc[:, :], in_=pg2[:, :])
                nc.gpsimd.dma_start(out=gin[:, 512 * qo:512 * qo + 512], in_=wkc[:, :])
            if NOCC:
                for c in range(NCORES):
                    nc.gpsimd.dma_start(out=gout[128 * c:128 * c + 128, :], in_=gin[:, :])
            else:
                nc.gpsimd.collective_compute("AllGather", A_.bypass,
                                             replica_groups=[list(range(NCORES))],
                                             ins=[gin[:, :].opt()], outs=[gout[:, :].opt()])
            # scatter gathered (w Kb)^T into wKT[q] ([ws; wt] global column order)
            for c in range(NCORES):
                for q in range(4):
                    nc.sync.dma_start(out=wKT[q][:, 256 * c:256 * c + 256],
                                      in_=gout[128 * c:128 * c + 128, 512 * q:512 * q + 256])
                    nc.scalar.dma_start(out=wKT[q][:, 2048 + 256 * c:2048 + 256 * c + 256],
                                        in_=gout[128 * c:128 * c + 128, 512 * q + 256:512 * q + 512])

            # ---------- noise prep ----------
            def noise_bcast(nin, tagp):
                cl = S.tile([1, 1], dt.float32, tag=f"cl{tagp}", name=f"cl{tagp}")
                nin_sb = S.tile([1, 1], dt.float32, tag=f"ni{tagp}", name=f"ni{tagp}")
                nc.sync.dma_start(out=nin_sb[:, :], in_=nin[:, :])
                nc.vector.tensor_scalar(out=cl[:, :], in0=nin_sb[:, :], scalar1=1e-5,
                                        scalar2=1.0, op0=A_.max, op1=A_.min)
                pn = PT.tile([128, 1], dt.float32, tag="pt", name="pt")
                nc.tensor.matmul(pn[:, :], onesr[:, :], cl[:, :], start=True, stop=True)
                nb = P.tile([128, 1], dt.float32, tag=f"nb{tagp}", name=f"nb{tagp}")
                nc.vector.tensor_copy(out=nb[:, :], in_=pn[:, :])
                return nb
            nbS = noise_bcast(noiseS, "S")
            nbT = noise_bcast(noiseT, "T")
            nsc = []
            for idx, (nsin, nb) in enumerate([(nsel0, nbS), (nsel1, nbS), (nsel2, nbT), (nsel3, nbT)]):
                t = S.tile([128, 16], dt.float32, tag=f"nselt{idx}", name=f"nselt{idx}")
                nc.sync.dma_start(out=t[:, :], in_=nsin[:, :])
                sc = P.tile([128, 16], dt.float32, tag=f"nsc{idx}", name=f"nsc{idx}")
                nc.vector.tensor_scalar(out=sc[:, :], in0=t[:, :], scalar1=nb[:, :],
                                        scalar2=None, op0=A_.mult)
                nsc.append(sc)

            # ---------- A build + pivot-gather prologue ----------
            nc.sync.dma_start(out=Arow[NBT][0:1, :], in_=yrow[0:1, 0:WSL])

            def build_A_row(i):
                pm = PW.tile([128, 512], dt.float32, tag="pw", name="pw")
                for qt in range(4):
                    nc.tensor.matmul(pm[:, :], wKT[qt][:, 128 * i:128 * i + 128],
                                     wlocb[qt][:, :], start=(qt == 0), stop=(qt == 3))
                kh = S.tile([128, 512], dt.float32, tag="kh", name="kh", bufs=3)
                nc.sync.dma_start(out=kh[:, :], in_=khad[128 * i:128 * i + 128, :])
                nc.vector.scalar_tensor_tensor(out=Arow[i][:, 0:512], in0=pm[:, :], scalar=1.0,
                                               in1=kh[:, :], op0=A_.mult, op1=A_.mult)
                if i < 16:
                    nc.vector.scalar_tensor_tensor(
                        out=Arow[i][:, 0:128], in0=Imask[:, :], scalar=nsc[0][:, i:i + 1],
                        in1=Arow[i][:, 0:128], op0=A_.mult, op1=A_.add)
                    nc.vector.scalar_tensor_tensor(
                        out=Arow[i][:, 128:256], in0=Imask[:, :], scalar=nsc[1][:, i:i + 1],
                        in1=Arow[i][:, 128:256], op0=A_.mult, op1=A_.add)
                else:
                    nc.vector.scalar_tensor_tensor(
                        out=Arow[i][:, 256:384], in0=Imask[:, :], scalar=nsc[2][:, i - 16:i - 15],
                        in1=Arow[i][:, 256:384], op0=A_.mult, op1=A_.add)
                    nc.vector.scalar_tensor_tensor(
                        out=Arow[i][:, 384:512], in0=Imask[:, :], scalar=nsc[3][:, i - 16:i - 15],
                        in1=Arow[i][:, 384:512], op0=A_.mult, op1=A_.add)
                nc.scalar.dma_start(out=Arow[i][:, 512:513], in_=ybcol[128 * i:128 * i + 128, :])

            def prologue(k):
                """push (already bf16-cast) pivot row k to DRAM, start its AllGather"""
                kk = k % 4
                if k == 0:
                    nc.vector.tensor_copy(out=rowb[kk][:, :], in_=Arow[k][:, :])
                ci = DR.tile([128, 512], dt.bfloat16, tag="cinb", name="cinb")
                nc.gpsimd.dma_start(out=ci[:, :], in_=rowb[kk][:, 0:512])
                co = DR.tile([1024, 512], dt.bfloat16, tag="coutb", name="coutb",
                             bufs=3, addr_space=SHARED)
                if NOCC:
                    for c in range(NCORES):
                        nc.gpsimd.dma_start(out=co[128 * c:128 * c + 128, :], in_=ci[:, :])
                else:
                    nc.gpsimd.collective_compute("AllGather", A_.bypass,
                                                 replica_groups=[list(range(NCORES))],
                                                 ins=[ci[:, :].opt()], outs=[co[:, :].opt()])
                return co

            for i in range(4):
                build_A_row(i)
            pend = {}
            if KSTEPS > 0:
                pend[0] = prologue(0)
            for i in range(4, NBT):
                build_A_row(i)

            st1.__exit__(None, None, None)

            # ---------- stage 2: elimination ----------
            st2 = tc.tile_pool(name="st2", bufs=1)
            P2 = st2.__enter__()
            Vb = [P2.tile([128, WSL], dt.bfloat16, tag=f"Vb{j}", name=f"Vb{j}") for j in range(4)]
            ld_acc = [P2.tile([128, 1], dt.float32, tag=f"ld{j}", name=f"ld{j}") for j in range(2)]
            nc.vector.memset(ld_acc[0][:, :], 0.0)
            ld_cur = 0
            cmid = P2.tile([1, 1], dt.float32, tag="cmid", name="cmid")
            cend = P2.tile([1, 1], dt.float32, tag="cend", name="cend")
            nc.vector.memset(cmid[:, :], 0.0)
            nc.vector.memset(cend[:, :], 0.0)

            def upd_row_single(r, kk, out_rowb=None):
                pu = PW.tile([128, WSL], dt.float32, tag="pw", name="pw")
                nc.tensor.matmul(pu[:, 0:512], grow[kk][:, f_off(r):f_off(r) + 128],
                                 Vb[kk][:, 0:512], start=True, stop=True)
                nc.tensor.matmul(pu[:, 512:513], grow[kk][:, f_off(r):f_off(r) + 128],
                                 Vb[kk][:, 512:513], start=True, stop=True)
                dst = Arow[r] if out_rowb is None else out_rowb
                nc.vector.scalar_tensor_tensor(out=dst[:, 0:512], in0=Arow[r][:, 0:512],
                                               scalar=1.0, in1=pu[:, 0:512],
                                               op0=A_.mult, op1=A_.subtract)
                nc.vector.scalar_tensor_tensor(out=dst[:, 512:513], in0=Arow[r][:, 512:513],
                                               scalar=1.0, in1=pu[:, 512:513],
                                               op0=A_.mult, op1=A_.subtract)

            def upd_row_fast(r, Mxb, src_rowb, out_rowb):
                """final update of the next pivot row via M = X G, writing bf16 slab"""
                pu = PW.tile([128, WSL], dt.float32, tag="pw", name="pw")
                nc.tensor.matmul(pu[:, 0:512], Mxb[:, :], src_rowb[:, 0:512], start=True, stop=True)
                nc.tensor.matmul(pu[:, 512:513], Mxb[:, :], src_rowb[:, 512:513], start=True, stop=True)
                nc.vector.scalar_tensor_tensor(out=out_rowb[:, 0:512], in0=Arow[r][:, 0:512],
                                               scalar=1.0, in1=pu[:, 0:512],
                                               op0=A_.mult, op1=A_.subtract)
                nc.vector.scalar_tensor_tensor(out=out_rowb[:, 512:513], in0=Arow[r][:, 512:513],
                                               scalar=1.0, in1=pu[:, 512:513],
                                               op0=A_.mult, op1=A_.subtract)

            def upd_row_batch(r, out_rowb=None):
                if r < NBT:
                    pu = PW.tile([128, WSL], dt.float32, tag="pw", name="pw")
                    for k2 in range(4):
                        nc.tensor.matmul(pu[:, 0:512], grow[k2][:, f_off(r):f_off(r) + 128],
                                         Vb[k2][:, 0:512], start=(k2 == 0), stop=(k2 == 3))
                    for k2 in range(4):
                        nc.tensor.matmul(pu[:, 512:513], grow[k2][:, f_off(r):f_off(r) + 128],
                                         Vb[k2][:, 512:513], start=(k2 == 0), stop=(k2 == 3))
                    dst = Arow[r] if out_rowb is None else out_rowb
                    nc.vector.scalar_tensor_tensor(out=dst[:, 0:512],
                                                   in0=Arow[r][:, 0:512], scalar=1.0,
                                                   in1=pu[:, 0:512], op0=A_.mult, op1=A_.subtract)
                    nc.vector.scalar_tensor_tensor(out=dst[:, 512:513],
                                                   in0=Arow[r][:, 512:513], scalar=1.0,
                                                   in1=pu[:, 512:513], op0=A_.mult, op1=A_.subtract)
                else:
                    pu = PT.tile([1, WSL], dt.float32, tag="pt", name="pt")
                    for k2 in range(4):
                        nc.tensor.matmul(pu[0:1, 0:512], rowb[k2][:, 512:513],
                                         Vb[k2][:, 0:512], start=(k2 == 0), stop=(k2 == 3))
                    for k2 in range(4):
                        nc.tensor.matmul(pu[0:1, 512:513], rowb[k2][:, 512:513],
                                         Vb[k2][:, 512:513], start=(k2 == 0), stop=(k2 == 3))
                    nc.vector.scalar_tensor_tensor(out=Arow[NBT][0:1, 0:512],
                                                   in0=Arow[NBT][0:1, 0:512], scalar=1.0,
                                                   in1=pu[0:1, 0:512], op0=A_.mult, op1=A_.subtract)
                    nc.vector.scalar_tensor_tensor(out=Arow[NBT][0:1, 512:513],
                                                   in0=Arow[NBT][0:1, 512:513], scalar=1.0,
                                                   in1=pu[0:1, 512:513], op0=A_.mult, op1=A_.subtract)

            for k in range(KSTEPS):
                kk = k % 4
                sup = k // 4
                co = pend.pop(k)
                # pull gathered row into SBUF; diag chunk first, immediate-update
                # chunks next, rest after
                c_dk = f_off(k) // 512
                order = [c_dk]
                for r in range(k + 1, 4 * sup + 4):
                    c = f_off(r) // 512
                    if c not in order:
                        order.append(c)
                for c in range(NCORES):
                    if c not in order:
                        order.append(c)
                for idx, c in enumerate(order):
                    eng = nc.sync if idx % 2 == 0 else nc.scalar
                    eng.dma_start(out=grow[kk][:, 512 * c:512 * c + 512],
                                  in_=co[128 * c:128 * c + 128, :])
                # pivot diag inverse via Newton-Schulz (bf16 matmuls; the inverse
                # is consumed as bf16 anyway)
                Dkb = grow[kk][:, f_off(k):f_off(k) + 128]
                scr = S.tile([128, 128], dt.float32, tag="scrD", name="scrD")
                dg = S.tile([128, 1], dt.float32, tag="dg", name="dg")
                nc.vector.tensor_tensor(out=scr[:, :], in0=Dkb, in1=Imask[:, :], op=A_.mult)
                nc.vector.tensor_reduce(out=dg[:, :], in_=scr[:, :],
                                        axis=mybir.AxisListType.X, op=A_.add)
                rcp = S.tile([128, 1], dt.float32, tag="rcp", name="rcp")
                nc.vector.reciprocal(out=rcp[:, :], in_=dg[:, :])
                X = S.tile([128, 128], dt.bfloat16, tag="Xns", name="Xns")
                nc.vector.tensor_scalar(out=X[:, :], in0=Imask[:, :], scalar1=rcp[:, :],
                                        scalar2=None, op0=A_.mult)
                pX = None
                for it in range(NS_ITERS):
                    pT = PQ.tile([128, 128], dt.float32, tag="psq", name="psq")
                    nc.tensor.matmul(pT[:, :], Dkb, X[:, :], start=True, stop=True)
                    Z = S.tile([128, 128], dt.bfloat16, tag="Zns", name="Zns")
                    nc.vector.scalar_tensor_tensor(out=Z[:, :], in0=Imask[:, :], scalar=2.0,
                                                   in1=pT[:, :], op0=A_.mult, op1=A_.subtract)
                    pX = PQ.tile([128, 128], dt.float32, tag="psq", name="psq")
                    nc.tensor.matmul(pX[:, :], X[:, :], Z[:, :], start=True, stop=True)
                    if it < NS_ITERS - 1:
                        X = S.tile([128, 128], dt.bfloat16, tag="Xns", name="Xns")
                        nc.vector.tensor_copy(out=X[:, :], in_=pX[:, :])
                INVb = S.tile([128, 128], dt.bfloat16, tag="INVb", name="INVb")
                nc.vector.tensor_copy(out=INVb[:, :], in_=pX[:, :])

                # critical path: final-update next pivot row via M = X G, then gather
                if kk < 3:
                    pMx = PQ.tile([128, 128], dt.float32, tag="psq", name="psq")
                    nc.tensor.matmul(pMx[:, :], INVb[:, :],
                                     grow[kk][:, f_off(k + 1):f_off(k + 1) + 128],
                                     start=True, stop=True)
                    Mxb = S.tile([128, 128], dt.bfloat16, tag="Mxb", name="Mxb")
                    nc.vector.tensor_copy(out=Mxb[:, :], in_=pMx[:, :])
                    upd_row_fast(k + 1, Mxb, rowb[kk], rowb[(k + 1) % 4])
                    pend[k + 1] = prologue(k + 1)

                # V = INV @ row_k (local slab width), for the remaining updates
                pv = PW.tile([128, WSL], dt.float32, tag="pw", name="pw")
                nc.tensor.matmul(pv[:, 0:512], INVb[:, :], rowb[kk][:, 0:512], start=True, stop=True)
                nc.tensor.matmul(pv[:, 512:513], INVb[:, :], rowb[kk][:, 512:513], start=True, stop=True)
                nc.vector.tensor_copy(out=Vb[kk][:, 0:512], in_=pv[:, 0:512])
                nc.vector.tensor_copy(out=Vb[kk][:, 512:513], in_=pv[:, 512:513])

                if kk < 3:
                    for r in range(k + 2, 4 * sup + 4):
                        upd_row_single(r, kk)
                else:
                    rows = list(range(4 * sup + 4, NBT + 1))
                    if k + 1 < KSTEPS:
                        upd_row_batch(rows[0], out_rowb=rowb[(k + 1) % 4])
                        pend[k + 1] = prologue(k + 1)
                        for r in rows[1:]:
                            upd_row_batch(r)
                    else:
                        for r in rows:
                            upd_row_batch(r)
                    if k == 15:
                        nc.vector.tensor_copy(out=cmid[:, :], in_=Arow[NBT][0:1, 512:513])
                    if k == KSTEPS - 1:
                        nc.vector.tensor_copy(out=cend[:, :], in_=Arow[NBT][0:1, 512:513])

                # logdet pieces (off the gather critical path)
                if k >= 16:
                    Dk = S.tile([128, 128], dt.float32, tag="Dk", name="Dk")
                    nc.vector.tensor_copy(out=Dk[:, :], in_=Dkb)
                    lg = S.tile([128, 1], dt.float32, tag="lg", name="lg")
                    nc.scalar.activation(out=lg[:, :], in_=dg[:, :], func=AF.Ln)
                    sq = S.tile([128, 1], dt.float32, tag="sq", name="sq")
                    nc.scalar.activation(out=sq[:, :], in_=rcp[:, :], func=AF.Sqrt)
                    T1 = S.tile([128, 128], dt.float32, tag="T1", name="T1")
                    nc.vector.tensor_scalar(out=T1[:, :], in0=Dk[:, :], scalar1=sq[:, :],
                                            scalar2=None, op0=A_.mult)
                    psr = PT.tile([1, 128], dt.float32, tag="pt", name="pt")
                    nc.tensor.matmul(psr[:, :], sq[:, :], Imask[:, :], start=True, stop=True)
                    sqr = S.tile([1, 128], dt.float32, tag="sqr", name="sqr")
                    nc.vector.tensor_copy(out=sqr[:, :], in_=psr[:, :])
                    pbc = PQ.tile([128, 128], dt.float32, tag="psq", name="psq")
                    nc.tensor.matmul(pbc[:, :], onesr[:, :], sqr[:, :], start=True, stop=True)
                    E1 = S.tile([128, 128], dt.float32, tag="E1", name="E1")
                    nc.vector.scalar_tensor_tensor(out=E1[:, :], in0=T1[:, :], scalar=1.0,
                                                   in1=pbc[:, :], op0=A_.mult, op1=A_.mult)
                    E = S.tile([128, 128], dt.float32, tag="Emat", name="Emat")
                    nc.vector.scalar_tensor_tensor(out=E[:, :], in0=E1[:, :], scalar=1.0,
                                                   in1=Imask[:, :], op0=A_.mult, op1=A_.subtract)
                    pows = [E]
                    # E2, E3, E4, E5
                    for (la, lb) in [(0, 0), (1, 0), (1, 1), (3, 0)]:
                        pp = PQ.tile([128, 128], dt.float32, tag="psq", name="psq")
                        nc.tensor.matmul(pp[:, :], pows[la][:, :], pows[lb][:, :],
                                         start=True, stop=True)
                        Ei = S.tile([128, 128], dt.float32, tag=f"E{len(pows) + 1}", name=f"E{len(pows) + 1}")
                        nc.vector.tensor_copy(out=Ei[:, :], in_=pp[:, :])
                        pows.append(Ei)
                    E2, E3, E4, E5 = pows[1], pows[2], pows[3], pows[4]
                    pairs = [(E, Imask, 1), (E, E, 2), (E2, E, 3), (E2, E2, 4), (E3, E2, 5),
                             (E3, E3, 6), (E4, E3, 7), (E4, E4, 8), (E5, E4, 9), (E5, E5, 10)]
                    ser = None
                    for (Pa, Pb, order_) in pairs:
                        scr2 = S.tile([128, 128], dt.float32, tag="scr2", name="scr2")
                        tr = S.tile([128, 1], dt.float32, tag=f"tr{order_}", name=f"tr{order_}")
                        nc.vector.tensor_tensor(out=scr2[:, :], in0=Pa[:, :], in1=Pb[:, :], op=A_.mult)
                        nc.vector.tensor_reduce(out=tr[:, :], in_=scr2[:, :],
                                                axis=mybir.AxisListType.X, op=A_.add)
                        coef = ((-1.0) ** (order_ + 1)) / order_
                        if ser is None:
                            ser = S.tile([128, 1], dt.float32, tag="ser", name="ser")
                            nc.vector.tensor_scalar(out=ser[:, :], in0=tr[:, :], scalar1=coef,
                                                    scalar2=None, op0=A_.mult)
                        else:
                            ser2 = S.tile([128, 1], dt.float32, tag="ser", name="ser")
                            nc.vector.scalar_tensor_tensor(out=ser2[:, :], in0=tr[:, :],
                                                           scalar=coef, in1=ser[:, :],
                                                           op0=A_.mult, op1=A_.add)
                            ser = ser2
                    tot = S.tile([128, 1], dt.float32, tag="totld", name="totld")
                    nc.vector.scalar_tensor_tensor(out=tot[:, :], in0=lg[:, :], scalar=1.0,
                                                   in1=ser[:, :], op0=A_.mult, op1=A_.add)
                    nxt = 1 - ld_cur
                    nc.vector.scalar_tensor_tensor(out=ld_acc[nxt][:, :], in0=tot[:, :],
                                                   scalar=1.0, in1=ld_acc[ld_cur][:, :],
                                                   op0=A_.mult, op1=A_.add)
                    ld_cur = nxt

            # ---------- finale ----------
            pld = PT.tile([1, 1], dt.float32, tag="pt", name="pt")
            nc.tensor.matmul(pld[:, :], ld_acc[ld_cur][:, :], onesc[:, :], start=True, stop=True)
            ldsum = S.tile([1, 1], dt.float32, tag="ldsum", name="ldsum")
            nc.vector.tensor_copy(out=ldsum[:, :], in_=pld[:, :])
            qd = S.tile([1, 1], dt.float32, tag="qd", name="qd")
            nc.vector.scalar_tensor_tensor(out=qd[:, :], in0=cmid[:, :], scalar=1.0,
                                           in1=cend[:, :], op0=A_.mult, op1=A_.subtract)
            b1 = S.tile([1, 1], dt.float32, tag="b1", name="b1")
            nc.vector.tensor_scalar(out=b1[:, :], in0=qd[:, :], scalar1=0.5,
                                    scalar2=CONST, op0=A_.mult, op1=A_.add)
            lossv = S.tile([1, 1], dt.float32, tag="lossv", name="lossv")
            nc.scalar.activation(out=lossv[:, :], in_=ldsum[:, :], func=AF.Identity,
                                 bias=b1[:, :], scale=0.25)
            nc.sync.dma_start(out=loss_out[:, :], in_=lossv[:, :])
            st2.__exit__(None, None, None)

    nc.compile()
    _NC_CACHE = nc
    return nc


LAST_EXEC_NS = None

def kernel(**inputs):
    global LAST_EXEC_NS
    from concourse.bass_utils import run_bass_kernel_spmd
    f32 = np.float32

    def arr(x):
        return np.ascontiguousarray(np.asarray(x, dtype=f32))

    sx, tx = arr(inputs["source_x"]), arr(inputs["target_x"])
    sy, ty = arr(inputs["source_y"]), arr(inputs["target_y"])
    k_ss, k_tt, k_st = arr(inputs["k_ss"]), arr(inputs["k_tt"]), arr(inputs["k_st"])
    Wf, bf = arr(inputs["Wf"]), arr(inputs["bf"])
    Ws, bs = arr(inputs["Ws"]), arr(inputs["bs"])
    Wt, bt = arr(inputs["Wt"]), arr(inputs["bt"])
    Kb = arr(inputs["Kb"])
    base_s, base_t = arr(inputs["base_s"]), arr(inputs["base_t"])
    noise_s, noise_t = arr(inputs["noise_s_opt"]), arr(inputs["noise_t_opt"])

    assert np.all(bf == 0.0), "kernel assumes bf == 0"
    ybcol = np.concatenate([sy[:, 0], ty[:, 0], np.zeros(128, f32)]).reshape(4224, 1).astype(f32)
    ones_row = np.ones((1, 128), f32)
    Imask = np.eye(128, dtype=f32)
    onescol = np.ones((128, 1), f32)
    WfT = np.ascontiguousarray(Wf.T)

    in_maps = []
    for c in range(NCORES):
        s0 = 256 * c
        nsel = [np.zeros((128, 16), f32) for _ in range(4)]
        nsel[0][:, 2 * c] = 1.0
        nsel[1][:, 2 * c + 1] = 1.0
        nsel[2][:, 2 * c] = 1.0
        nsel[3][:, 2 * c + 1] = 1.0
        khad_c = np.empty((4096, 512), f32)
        khad_c[0:2048, 0:256] = k_ss[:, s0:s0 + 256]
        khad_c[0:2048, 256:512] = k_st[:, s0:s0 + 256]
        khad_c[2048:4096, 0:256] = k_st[s0:s0 + 256, :].T
        khad_c[2048:4096, 256:512] = k_tt[:, s0:s0 + 256]
        yrow = np.zeros((1, WSL), f32)
        yrow[0, 0:256] = sy[s0:s0 + 256, 0]
        yrow[0, 256:512] = ty[s0:s0 + 256, 0]
        in_maps.append(dict(
            xsT=np.ascontiguousarray(sx[s0:s0 + 256, :].T),
            xtT=np.ascontiguousarray(tx[s0:s0 + 256, :].T),
            bsT=np.ascontiguousarray(base_s.T),
            btT=np.ascontiguousarray(base_t.T),
            WfT=WfT,
            WaS=np.tile(Ws[0:1, 0:8], (128, 1)).astype(f32),
            WaT=np.tile(Wt[0:1, 0:8], (128, 1)).astype(f32),
            WcS=np.ascontiguousarray(Ws[0, 8:16].reshape(8, 1)),
            WcT=np.ascontiguousarray(Wt[0, 8:16].reshape(8, 1)),
            bS=bs.reshape(1, 1), bT=bt.reshape(1, 1),
            noiseS=noise_s.reshape(1, 1), noiseT=noise_t.reshape(1, 1),
            Kb=Kb, khad=khad_c, ybcol=ybcol, yrow=yrow,
            ones_row=ones_row, Imask=Imask, onescol=onescol,
            nsel0=nsel[0], nsel1=nsel[1], nsel2=nsel[2], nsel3=nsel[3],
        ))

    nc = build()
    trace = bool(int(os.environ.get("KERNEL_TRACE", "0")))
    loss = None
    if os.environ.get("KERNEL_FORCE_SIM", "0") != "1":
        try:
            kw = {}
            td = os.environ.get("KERNEL_TRACE_DIR")
            if td:
                kw["tmpdir"] = td
            res = run_bass_kernel_spmd(nc, in_maps, core_ids=list(range(NCORES)), trace=trace, **kw)
            LAST_EXEC_NS = res.exec_time_ns
            loss = np.float32(res.results[0]["loss"][0, 0])
        except Exception as e:
            sys.stderr.write("HW path failed (%s); falling back to MultiCoreSim\n" % type(e).__name__)
    if loss is None or not np.isfinite(loss):
        from concourse.bass_interp import MultiCoreSim
        sim = MultiCoreSim(nc, num_cores=NCORES, trace=False,
                           require_finite=False, require_nnan=False)
        for i in range(NCORES):
            for kk, vv in in_maps[i].items():
                sim.cores[i].tensor(kk)[:] = vv
        sim.simulate(check_with_hw=False)
        loss = np.float32(sim.cores[0].mem_tensor("loss")[0, 0])
    return np.asarray(loss, dtype=np.float32).reshape(())


# revision 16
# speedup vs baseline: 1.2833x; 1.0930x over previous
import sys, os
sys.path.insert(0, "/opt/trn_rl_repo")
import numpy as np

NCORES = 8
B = 128           # block size
NBT = 32          # matrix block-rows (4096/128)
WSL = 513         # slab width: 512 matrix cols + 1 border (y) col
NS_ITERS = 3
KSTEPS = int(os.environ.get('KSTEPS', '32'))
NOCC = os.environ.get('NOCC', '0') == '1'
CONST = 0.5 * 2048 * float(np.log(2 * np.pi))

# Block-cyclic column sharding: core c owns global blocks {c, 8+c, 16+c, 24+c}
# (positions 0..3 in its local slab). Position p dies for all cores at the
# epoch boundary k = 8(p+1), so the active slab width shrinks uniformly:
# epoch e = k//8, W0 = 128e, active local cols = [W0, 512) plus the border col.

def owner(r):
    return r % 8

def pos(r):
    return r // 8


_NC_CACHE = None

def build():
    global _NC_CACHE
    if _NC_CACHE is not None:
        return _NC_CACHE
    import concourse.bass as bass
    import concourse.bacc as bacc
    import concourse.tile as tile
    import concourse.mybir as mybir
    dt = mybir.dt
    A_ = mybir.AluOpType
    AF = mybir.ActivationFunctionType
    SHARED = "Local" if NOCC else "Shared"

    nc = bacc.Bacc("TRN2", target_bir_lowering=False, debug=False, num_devices=NCORES)
    def din(name, shape):
        return nc.dram_tensor(name, shape, dt.float32, kind="ExternalInput").ap()
    xsT = din("xsT", [256, 256]);  xtT = din("xtT", [256, 256])
    bsT = din("bsT", [256, 512]);  btT = din("btT", [256, 512])
    WfT = din("WfT", [256, 8])
    WaS = din("WaS", [128, 8]);    WaT = din("WaT", [128, 8])
    WcS = din("WcS", [8, 1]);      WcT = din("WcT", [8, 1])
    bS = din("bS", [1, 1]);        bT = din("bT", [1, 1])
    noiseS = din("noiseS", [1, 1]); noiseT = din("noiseT", [1, 1])
    Kb_in = din("Kb", [512, 512])
    khad = din("khad", [4096, 512])
    ybcol = din("ybcol", [4224, 1])
    yrow = din("yrow", [1, WSL])
    ones_row = din("ones_row", [1, 128])
    Imask_in = din("Imask", [128, 128])
    onescol = din("onescol", [128, 1])
    nsel_in = din("nsel", [128, 8])
    loss_out = nc.dram_tensor("loss", [1, 1], dt.float32, kind="ExternalOutput").ap()

    with tile.TileContext(nc) as tc:
        with tc.tile_pool(name="pers", bufs=1) as P, \
             tc.tile_pool(name="strm", bufs=2) as S, \
             tc.tile_pool(name="pw", bufs=2, space="PSUM") as PW, \
             tc.tile_pool(name="psq", bufs=2, space="PSUM") as PQ, \
             tc.tile_pool(name="pt", bufs=1, space="PSUM") as PT, \
             tc.tile_pool(name="dram", bufs=2, space="DRAM") as DR:

            # ---------- constants into SBUF ----------
            Imask = P.tile([128, 128], dt.float32, tag="Imask", name="Imask")
            nc.sync.dma_start(out=Imask[:, :], in_=Imask_in[:, :])
            onesr = P.tile([1, 128], dt.float32, tag="onesr", name="onesr")
            nc.sync.dma_start(out=onesr[:, :], in_=ones_row[:, :])
            onesc = P.tile([128, 1], dt.float32, tag="onesc", name="onesc")
            nc.sync.dma_start(out=onesc[:, :], in_=onescol[:, :])

            # ---------- persistent stage-2 state ----------
            Arow = [P.tile([128, WSL], dt.float32, tag=f"A{i}", name=f"A{i}") for i in range(NBT)]
            Arow.append(P.tile([1, WSL], dt.float32, tag="A32", name="A32"))
            rowb = [P.tile([128, WSL], dt.bfloat16, tag=f"rowb{j}", name=f"rowb{j}") for j in range(4)]
            grow = [P.tile([128, 4096], dt.bfloat16, tag=f"grow{j}", name=f"grow{j}") for j in range(4)]
            Vb = [P.tile([128, WSL], dt.bfloat16, tag=f"Vb{j}", name=f"Vb{j}") for j in range(4)]
            ld_acc = [P.tile([128, 1], dt.float32, tag=f"ld{j}", name=f"ld{j}") for j in range(2)]
            nc.vector.memset(ld_acc[0][:, :], 0.0)
            ld_cur = 0
            cmid = P.tile([1, 1], dt.float32, tag="cmid", name="cmid")
            cend = P.tile([1, 1], dt.float32, tag="cend", name="cend")
            nc.vector.memset(cmid[:, :], 0.0)
            nc.vector.memset(cend[:, :], 0.0)

            # ---------- stage 1 scratch ----------
            st1 = tc.tile_pool(name="st1", bufs=1)
            P1 = st1.__enter__()
            wlocb = [P1.tile([128, 512], dt.bfloat16, tag=f"wlocb{q}", name=f"wlocb{q}") for q in range(4)]
            Kbb = [P1.tile([128, 512], dt.bfloat16, tag=f"Kbb{j}", name=f"Kbb{j}") for j in range(4)]
            wKT = [P1.tile([128, 4096], dt.bfloat16, tag=f"wKT{q}", name=f"wKT{q}") for q in range(4)]

            # Kb loads early so the DMA overlaps the w build
            for j in range(4):
                kbt = S.tile([128, 512], dt.float32, tag="kbt", name="kbt")
                nc.sync.dma_start(out=kbt[:, :], in_=Kb_in[128 * j:128 * j + 128, :])
                nc.vector.tensor_copy(out=Kbb[j][:, :], in_=kbt[:, :])

            # ---------- stage 1: w build (writes wlocb directly) ----------
            def build_w(xT, bT_, Wa_in, Wc, bias, m_off):
                xt = [S.tile([128, 256], dt.float32, tag="xt", name="xt") for _ in range(2)]
                nc.sync.dma_start(out=xt[0][:, :], in_=xT[0:128, :])
                nc.sync.dma_start(out=xt[1][:, :], in_=xT[128:256, :])
                bt_ = [S.tile([128, 512], dt.float32, tag="bt", name="bt") for _ in range(2)]
                nc.sync.dma_start(out=bt_[0][:, :], in_=bT_[0:128, :])
                nc.sync.dma_start(out=bt_[1][:, :], in_=bT_[128:256, :])
                wft = [S.tile([128, 8], dt.float32, tag="wft", name="wft") for _ in range(2)]
                nc.sync.dma_start(out=wft[0][:, :], in_=WfT[0:128, :])
                nc.sync.dma_start(out=wft[1][:, :], in_=WfT[128:256, :])
                wa = S.tile([128, 8], dt.float32, tag="wa", name="wa")
                nc.sync.dma_start(out=wa[:, :], in_=Wa_in[:, :])
                wc = S.tile([8, 1], dt.float32, tag="wc", name="wc")
                nc.sync.dma_start(out=wc[:, :], in_=Wc[:, :])
                bia = S.tile([1, 1], dt.float32, tag="bia", name="bia")
                nc.sync.dma_start(out=bia[:, :], in_=bias[:, :])

                # gT = Wf @ base^T : [8, 512]
                pg = PW.tile([8, 512], dt.float32, tag="pw", name="pw")
                for kc in range(2):
                    nc.tensor.matmul(pg[:, :], wft[kc][:, :], bt_[kc][:, :],
                                     start=(kc == 0), stop=(kc == 1))
                gT = S.tile([8, 512], dt.float32, tag="gT", name="gT")
                nc.vector.tensor_copy(out=gT[:, :], in_=pg[:, :])
                # crow = Wc^T @ gT + bias : [1, 512]
                pc = PT.tile([1, 512], dt.float32, tag="pt", name="pt")
                nc.tensor.matmul(pc[:, :], wc[:, :], gT[:, :], start=True, stop=True)
                crow = S.tile([1, 512], dt.float32, tag="crow", name="crow")
                nc.scalar.activation(out=crow[:, :], in_=pc[:, :], func=AF.Identity,
                                     bias=bia[:, :], scale=1.0)
                # bcast crow to 128 partitions
                pb = PW.tile([128, 512], dt.float32, tag="pw", name="pw")
                nc.tensor.matmul(pb[:, :], onesr[:, :], crow[:, :], start=True, stop=True)

                for r in range(2):
                    pf = PQ.tile([128, 8], dt.float32, tag="psq", name="psq")
                    for kc in range(2):
                        nc.tensor.matmul(pf[:, :], xt[kc][:, 128 * r:128 * r + 128],
                                         wft[kc][:, :], start=(kc == 0), stop=(kc == 1))
                    f_r = S.tile([128, 8], dt.float32, tag="f_r", name="f_r")
                    nc.vector.tensor_copy(out=f_r[:, :], in_=pf[:, :])
                    scr8 = S.tile([128, 8], dt.float32, tag="scr8", name="scr8")
                    a_r = S.tile([128, 1], dt.float32, tag="a_r", name="a_r")
                    nc.vector.tensor_tensor(out=scr8[:, :], in0=f_r[:, :], in1=wa[:, :], op=A_.mult)
                    nc.vector.tensor_reduce(out=a_r[:, :], in_=scr8[:, :],
                                            axis=mybir.AxisListType.X, op=A_.add)
                    raw = S.tile([128, 512], dt.float32, tag="raw", name="raw", bufs=1)
                    nc.vector.tensor_scalar(out=raw[:, :], in0=pb[:, :], scalar1=a_r[:, :],
                                            scalar2=None, op0=A_.add)
                    mn = S.tile([128, 512], dt.float32, tag="mn", name="mn", bufs=1)
                    nc.vector.tensor_scalar(out=mn[:, :], in0=raw[:, :], scalar1=0.0,
                                            scalar2=None, op0=A_.min)
                    ex = S.tile([128, 512], dt.float32, tag="ex", name="ex", bufs=1)
                    nc.scalar.activation(out=ex[:, :], in_=mn[:, :], func=AF.Exp)
                    mx = S.tile([128, 512], dt.float32, tag="mx", name="mx", bufs=1)
                    nc.vector.tensor_scalar(out=mx[:, :], in0=raw[:, :], scalar1=0.0,
                                            scalar2=None, op0=A_.max)
                    w_r = S.tile([128, 512], dt.float32, tag="w_r", name="w_r")
                    nc.vector.scalar_tensor_tensor(out=w_r[:, :], in0=ex[:, :], scalar=-1.0,
                                                   in1=mx[:, :], op0=A_.add, op1=A_.add)
                    sm = S.tile([128, 1], dt.float32, tag="sm", name="sm")
                    nc.vector.tensor_reduce(out=sm[:, :], in_=w_r[:, :],
                                            axis=mybir.AxisListType.X, op=A_.add)
                    rec = S.tile([128, 1], dt.float32, tag="rec", name="rec")
                    nc.vector.reciprocal(out=rec[:, :], in_=sm[:, :])
                    nc.vector.tensor_scalar(out=w_r[:, :], in0=w_r[:, :], scalar1=rec[:, :],
                                            scalar2=512.0, op0=A_.mult, op1=A_.mult)
                    # transpose 128x128 chunks straight into bf16 wlocb
                    for q in range(4):
                        ptr = PQ.tile([128, 128], dt.float32, tag="psq", name="psq")
                        nc.tensor.matmul(ptr[:, :], w_r[:, 128 * q:128 * q + 128], Imask[:, :], start=True, stop=True)
                        nc.vector.tensor_copy(
                            out=wlocb[q][:, m_off + 128 * r: m_off + 128 * r + 128],
                            in_=ptr[:, :])

            build_w(xsT, bsT, WaS, WcS, bS, 0)
            build_w(xtT, btT, WaT, WcT, bT, 256)

            # ---------- local (w Kb)^T chunks, single bf16 gather ----------
            gin = DR.tile([128, 2048], dt.bfloat16, tag="gin", name="gin")
            gout = DR.tile([1024, 2048], dt.bfloat16, tag="gout", name="gout",
                           addr_space=SHARED)
            for qo in range(4):
                pg2 = PW.tile([128, 512], dt.float32, tag="pw", name="pw")
                for jt in range(4):
                    nc.tensor.matmul(pg2[:, :], Kbb[jt][:, 128 * qo:128 * qo + 128],
                                     wlocb[jt][:, :], start=(jt == 0), stop=(jt == 3))
                wkc = S.tile([128, 512], dt.bfloat16, tag="wkc", name="wkc")
                nc.vector.tensor_copy(out=wkc[:, :], in_=pg2[:, :])
                nc.gpsimd.dma_start(out=gin[:, 512 * qo:512 * qo + 512], in_=wkc[:, :])
            if NOCC:
                for c in range(NCORES):
                    nc.gpsimd.dma_start(out=gout[128 * c:128 * c + 128, :], in_=gin[:, :])
            else:
                nc.gpsimd.collective_compute("AllGather", A_.bypass,
                                             replica_groups=[list(range(NCORES))],
                                             ins=[gin[:, :].opt()], outs=[gout[:, :].opt()])
            # scatter gathered (w Kb)^T into wKT[q]; core c's local matrix cols map
            # to global w-row blocks {c, 8+c} (source) and {16+c, 24+c} (target)
            for c in range(NCORES):
                for q in range(4):
                    for lp, gblk in [(0, c), (1, 8 + c), (2, 16 + c), (3, 24 + c)]:
                        nc.sync.dma_start(
                            out=wKT[q][:, 128 * gblk:128 * gblk + 128],
                            in_=gout[128 * c:128 * c + 128, 512 * q + 128 * lp:512 * q + 128 * lp + 128])

            # ---------- noise prep ----------
            def noise_bcast(nin, tagp):
                cl = S.tile([1, 1], dt.float32, tag=f"cl{tagp}", name=f"cl{tagp}")
                nin_sb = S.tile([1, 1], dt.float32, tag=f"ni{tagp}", name=f"ni{tagp}")
                nc.sync.dma_start(out=nin_sb[:, :], in_=nin[:, :])
                nc.vector.tensor_scalar(out=cl[:, :], in0=nin_sb[:, :], scalar1=1e-5,
                                        scalar2=1.0, op0=A_.max, op1=A_.min)
                pn = PT.tile([128, 1], dt.float32, tag="pt", name="pt")
                nc.tensor.matmul(pn[:, :], onesr[:, :], cl[:, :], start=True, stop=True)
                nb = P.tile([128, 1], dt.float32, tag=f"nb{tagp}", name=f"nb{tagp}")
                nc.vector.tensor_copy(out=nb[:, :], in_=pn[:, :])
                return nb
            nbS = noise_bcast(noiseS, "S")
            nbT = noise_bcast(noiseT, "T")
            nselt = S.tile([128, 8], dt.float32, tag="nselt", name="nselt")
            nc.sync.dma_start(out=nselt[:, :], in_=nsel_in[:, :])
            nscS = P.tile([128, 8], dt.float32, tag="nscS", name="nscS")
            nc.vector.tensor_scalar(out=nscS[:, :], in0=nselt[:, :], scalar1=nbS[:, :],
                                    scalar2=None, op0=A_.mult)
            nscT = P.tile([128, 8], dt.float32, tag="nscT", name="nscT")
            nc.vector.tensor_scalar(out=nscT[:, :], in0=nselt[:, :], scalar1=nbT[:, :],
                                    scalar2=None, op0=A_.mult)

            # ---------- A build + pivot-gather prologue ----------
            nc.sync.dma_start(out=Arow[NBT][0:1, :], in_=yrow[0:1, 0:WSL])

            def build_A_row(i):
                pm = PW.tile([128, 512], dt.float32, tag="pw", name="pw")
                for qt in range(4):
                    nc.tensor.matmul(pm[:, :], wKT[qt][:, 128 * i:128 * i + 128],
                                     wlocb[qt][:, :], start=(qt == 0), stop=(qt == 3))
                kh = S.tile([128, 512], dt.float32, tag="kh", name="kh", bufs=3)
                nc.sync.dma_start(out=kh[:, :], in_=khad[128 * i:128 * i + 128, :])
                nc.vector.scalar_tensor_tensor(out=Arow[i][:, 0:512], in0=pm[:, :], scalar=1.0,
                                               in1=kh[:, :], op0=A_.mult, op1=A_.mult)
                # diagonal noise: row block i's diag sits at local position i//8 on
                # the owner core (mask input selects the owner)
                p = pos(i)
                nsc = nscS if i < 16 else nscT
                j = i % 8
                nc.vector.scalar_tensor_tensor(
                    out=Arow[i][:, 128 * p:128 * p + 128], in0=Imask[:, :],
                    scalar=nsc[:, j:j + 1],
                    in1=Arow[i][:, 128 * p:128 * p + 128], op0=A_.mult, op1=A_.add)
                nc.scalar.dma_start(out=Arow[i][:, 512:513], in_=ybcol[128 * i:128 * i + 128, :])

            def prologue(k):
                """push (already bf16-cast) pivot row k to DRAM, start its AllGather"""
                kk = k % 4
                W0 = 128 * (k // 8)
                w = 512 - W0
                if k == 0:
                    nc.vector.tensor_copy(out=rowb[kk][:, :], in_=Arow[k][:, :])
                ci = DR.tile([128, w], dt.bfloat16, tag=f"cinb{k // 8}", name="cinb")
                nc.gpsimd.dma_start(out=ci[:, :], in_=rowb[kk][:, W0:512])
                co = DR.tile([1024, w], dt.bfloat16, tag=f"coutb{k // 8}", name="coutb",
                             bufs=3, addr_space=SHARED)
                if NOCC:
                    for c in range(NCORES):
                        nc.gpsimd.dma_start(out=co[128 * c:128 * c + 128, :], in_=ci[:, :])
                else:
                    nc.gpsimd.collective_compute("AllGather", A_.bypass,
                                                 replica_groups=[list(range(NCORES))],
                                                 ins=[ci[:, :].opt()], outs=[co[:, :].opt()])
                return co

            for i in range(4):
                build_A_row(i)
            pend = {}
            if KSTEPS > 0:
                pend[0] = prologue(0)
            for i in range(4, 14):
                build_A_row(i)
            # rows 14..31 are emitted inside the first pivot cycles (PE fills the
            # gather windows); schedule: 6 rows after each of pivots 0,1,2
            arow_sched = {0: range(14, 20), 1: range(20, 26), 2: range(26, 32)}

            # ---------- stage 2: elimination ----------
            def upd_row_single(r, kk, W0):
                """immediate update of row r by pivot kk (within superstep)"""
                w = 512 - W0
                woff = lambda rr: w * owner(rr) + 128 * pos(rr) - W0
                pu = PW.tile([128, WSL], dt.float32, tag="pw", name="pw")
                g = grow[kk][:, woff(r):woff(r) + 128]
                nc.tensor.matmul(pu[:, W0:512], g, Vb[kk][:, W0:512], start=True, stop=True)
                nc.tensor.matmul(pu[:, 512:513], g, Vb[kk][:, 512:513], start=True, stop=True)
                nc.vector.scalar_tensor_tensor(out=Arow[r][:, W0:512], in0=Arow[r][:, W0:512],
                                               scalar=1.0, in1=pu[:, W0:512],
                                               op0=A_.mult, op1=A_.subtract)
                nc.vector.scalar_tensor_tensor(out=Arow[r][:, 512:513], in0=Arow[r][:, 512:513],
                                               scalar=1.0, in1=pu[:, 512:513],
                                               op0=A_.mult, op1=A_.subtract)

            def upd_row_fast(r, Mxb, src_rowb, out_rowb, W0):
                """final update of the next pivot row via M = X G, writing bf16 slab"""
                pu = PW.tile([128, WSL], dt.float32, tag="pw", name="pw")
                nc.tensor.matmul(pu[:, W0:512], Mxb[:, :], src_rowb[:, W0:512], start=True, stop=True)
                nc.tensor.matmul(pu[:, 512:513], Mxb[:, :], src_rowb[:, 512:513], start=True, stop=True)
                nc.vector.scalar_tensor_tensor(out=out_rowb[:, W0:512], in0=Arow[r][:, W0:512],
                                               scalar=1.0, in1=pu[:, W0:512],
                                               op0=A_.mult, op1=A_.subtract)
                nc.vector.scalar_tensor_tensor(out=out_rowb[:, 512:513], in0=Arow[r][:, 512:513],
                                               scalar=1.0, in1=pu[:, 512:513],
                                               op0=A_.mult, op1=A_.subtract)

            def upd_row_batch(r, W0, out_rowb=None):
                w = 512 - W0
                woff = lambda rr: w * owner(rr) + 128 * pos(rr) - W0
                if r < NBT:
                    pu = PW.tile([128, WSL], dt.float32, tag="pw", name="pw")
                    for k2 in range(4):
                        nc.tensor.matmul(pu[:, W0:512], grow[k2][:, woff(r):woff(r) + 128],
                                         Vb[k2][:, W0:512], start=(k2 == 0), stop=(k2 == 3))
                    for k2 in range(4):
                        nc.tensor.matmul(pu[:, 512:513], grow[k2][:, woff(r):woff(r) + 128],
                                         Vb[k2][:, 512:513], start=(k2 == 0), stop=(k2 == 3))
                    dst = Arow[r] if out_rowb is None else out_rowb
                    nc.vector.scalar_tensor_tensor(out=dst[:, W0:512],
                                                   in0=Arow[r][:, W0:512], scalar=1.0,
                                                   in1=pu[:, W0:512], op0=A_.mult, op1=A_.subtract)
                    nc.vector.scalar_tensor_tensor(out=dst[:, 512:513],
                                                   in0=Arow[r][:, 512:513], scalar=1.0,
                                                   in1=pu[:, 512:513], op0=A_.mult, op1=A_.subtract)
                else:
                    pu = PT.tile([1, WSL], dt.float32, tag="pt", name="pt")
                    for k2 in range(4):
                        nc.tensor.matmul(pu[0:1, W0:512], rowb[k2][:, 512:513],
                                         Vb[k2][:, W0:512], start=(k2 == 0), stop=(k2 == 3))
                    for k2 in range(4):
                        nc.tensor.matmul(pu[0:1, 512:513], rowb[k2][:, 512:513],
                                         Vb[k2][:, 512:513], start=(k2 == 0), stop=(k2 == 3))
                    nc.vector.scalar_tensor_tensor(out=Arow[NBT][0:1, W0:512],
                                                   in0=Arow[NBT][0:1, W0:512], scalar=1.0,
                                                   in1=pu[0:1, W0:512], op0=A_.mult, op1=A_.subtract)
                    nc.vector.scalar_tensor_tensor(out=Arow[NBT][0:1, 512:513],
                                                   in0=Arow[NBT][0:1, 512:513], scalar=1.0,
                                                   in1=pu[0:1, 512:513], op0=A_.mult, op1=A_.subtract)

            for k in range(KSTEPS):
                kk = k % 4
                sup = k // 4
                W0 = 128 * (k // 8)
                w = 512 - W0
                co = pend.pop(k)
                # pull gathered row into SBUF; diag chunk first, immediate-update
                # chunks next, rest after
                order = [owner(k)]
                for r in range(k + 1, 4 * sup + 4):
                    if owner(r) not in order:
                        order.append(owner(r))
                for c in range(NCORES):
                    if c not in order:
                        order.append(c)
                for idx, c in enumerate(order):
                    eng = nc.sync if idx % 2 == 0 else nc.scalar
                    eng.dma_start(out=grow[kk][:, w * c:w * c + w],
                                  in_=co[128 * c:128 * c + 128, :])
                # pivot diag inverse via Newton-Schulz (bf16 matmuls; the inverse
                # is consumed as bf16 anyway)
                Dkb = grow[kk][:, w * owner(k):w * owner(k) + 128]
                scr = S.tile([128, 128], dt.float32, tag="scrD", name="scrD")
                dg = S.tile([128, 1], dt.float32, tag="dg", name="dg")
                nc.vector.tensor_tensor(out=scr[:, :], in0=Dkb, in1=Imask[:, :], op=A_.mult)
                nc.vector.tensor_reduce(out=dg[:, :], in_=scr[:, :],
                                        axis=mybir.AxisListType.X, op=A_.add)
                rcp = S.tile([128, 1], dt.float32, tag="rcp", name="rcp")
                nc.vector.reciprocal(out=rcp[:, :], in_=dg[:, :])
                X = S.tile([128, 128], dt.bfloat16, tag="Xns", name="Xns")
                nc.vector.tensor_scalar(out=X[:, :], in0=Imask[:, :], scalar1=rcp[:, :],
                                        scalar2=None, op0=A_.mult)
                pX = None
                for it in range(NS_ITERS):
                    pT = PQ.tile([128, 128], dt.float32, tag="psq", name="psq")
                    nc.tensor.matmul(pT[:, :], Dkb, X[:, :], start=True, stop=True)
                    Z = S.tile([128, 128], dt.bfloat16, tag="Zns", name="Zns")
                    nc.vector.scalar_tensor_tensor(out=Z[:, :], in0=Imask[:, :], scalar=2.0,
                                                   in1=pT[:, :], op0=A_.mult, op1=A_.subtract)
                    pX = PQ.tile([128, 128], dt.float32, tag="psq", name="psq")
                    nc.tensor.matmul(pX[:, :], X[:, :], Z[:, :], start=True, stop=True)
                    if it < NS_ITERS - 1:
                        X = S.tile([128, 128], dt.bfloat16, tag="Xns", name="Xns")
                        nc.vector.tensor_copy(out=X[:, :], in_=pX[:, :])
                INVb = S.tile([128, 128], dt.bfloat16, tag="INVb", name="INVb")
                nc.vector.tensor_copy(out=INVb[:, :], in_=pX[:, :])

                # critical path: final-update next pivot row via M = X G, then gather
                if kk < 3:
                    r1 = k + 1
                    goff = w * owner(r1) + 128 * pos(r1) - W0
                    pMx = PQ.tile([128, 128], dt.float32, tag="psq", name="psq")
                    nc.tensor.matmul(pMx[:, :], INVb[:, :], grow[kk][:, goff:goff + 128],
                                     start=True, stop=True)
                    Mxb = S.tile([128, 128], dt.bfloat16, tag="Mxb", name="Mxb")
                    nc.vector.tensor_copy(out=Mxb[:, :], in_=pMx[:, :])
                    upd_row_fast(r1, Mxb, rowb[kk], rowb[r1 % 4], W0)
                    pend[r1] = prologue(r1)

                # V = INV @ row_k (active slab width), for the remaining updates
                pv = PW.tile([128, WSL], dt.float32, tag="pw", name="pw")
                nc.tensor.matmul(pv[:, W0:512], INVb[:, :], rowb[kk][:, W0:512], start=True, stop=True)
                nc.tensor.matmul(pv[:, 512:513], INVb[:, :], rowb[kk][:, 512:513], start=True, stop=True)
                nc.vector.tensor_copy(out=Vb[kk][:, W0:512], in_=pv[:, W0:512])
                nc.vector.tensor_copy(out=Vb[kk][:, 512:513], in_=pv[:, 512:513])

                # deferred A-build rows fill this pivot's gather window
                for i in arow_sched.get(k, []):
                    build_A_row(i)
                if k == 2:
                    st1.__exit__(None, None, None)

                if kk < 3:
                    for r in range(k + 2, 4 * sup + 4):
                        upd_row_single(r, kk, W0)
                else:
                    rows = list(range(4 * sup + 4, NBT + 1))
                    if k + 1 < KSTEPS:
                        upd_row_batch(rows[0], W0, out_rowb=rowb[(k + 1) % 4])
                        pend[k + 1] = prologue(k + 1)
                        for r in rows[1:]:
                            upd_row_batch(r, W0)
                    else:
                        for r in rows:
                            upd_row_batch(r, W0)
                    if k == 15:
                        nc.vector.tensor_copy(out=cmid[:, :], in_=Arow[NBT][0:1, 512:513])
                    if k == KSTEPS - 1:
                        nc.vector.tensor_copy(out=cend[:, :], in_=Arow[NBT][0:1, 512:513])

                # logdet pieces (off the gather critical path)
                if k >= 16:
                    Dk = S.tile([128, 128], dt.float32, tag="Dk", name="Dk")
                    nc.vector.tensor_copy(out=Dk[:, :], in_=Dkb)
                    lg = S.tile([128, 1], dt.float32, tag="lg", name="lg")
                    nc.scalar.activation(out=lg[:, :], in_=dg[:, :], func=AF.Ln)
                    sq = S.tile([128, 1], dt.float32, tag="sq", name="sq")
                    nc.scalar.activation(out=sq[:, :], in_=rcp[:, :], func=AF.Sqrt)
                    T1 = S.tile([128, 128], dt.float32, tag="T1", name="T1")
                    nc.vector.tensor_scalar(out=T1[:, :], in0=Dk[:, :], scalar1=sq[:, :],
                                            scalar2=None, op0=A_.mult)
                    psr = PT.tile([1, 128], dt.float32, tag="pt", name="pt")
                    nc.tensor.matmul(psr[:, :], sq[:, :], Imask[:, :], start=True, stop=True)
                    sqr = S.tile([1, 128], dt.float32, tag="sqr", name="sqr")
                    nc.vector.tensor_copy(out=sqr[:, :], in_=psr[:, :])
                    pbc = PQ.tile([128, 128], dt.float32, tag="psq", name="psq")
                    nc.tensor.matmul(pbc[:, :], onesr[:, :], sqr[:, :], start=True, stop=True)
                    E1 = S.tile([128, 128], dt.float32, tag="E1", name="E1")
                    nc.vector.scalar_tensor_tensor(out=E1[:, :], in0=T1[:, :], scalar=1.0,
                                                   in1=pbc[:, :], op0=A_.mult, op1=A_.mult)
                    E = S.tile([128, 128], dt.float32, tag="Emat", name="Emat")
                    nc.vector.scalar_tensor_tensor(out=E[:, :], in0=E1[:, :], scalar=1.0,
                                                   in1=Imask[:, :], op0=A_.mult, op1=A_.subtract)
                    pows = [E]
                    # E2, E3, E4, E5
                    for (la, lb) in [(0, 0), (1, 0), (1, 1), (3, 0)]:
                        pp = PQ.tile([128, 128], dt.float32, tag="psq", name="psq")
                        nc.tensor.matmul(pp[:, :], pows[la][:, :], pows[lb][:, :],
                                         start=True, stop=True)
                        Ei = S.tile([128, 128], dt.float32, tag=f"E{len(pows) + 1}", name=f"E{len(pows) + 1}")
                        nc.vector.tensor_copy(out=Ei[:, :], in_=pp[:, :])
                        pows.append(Ei)
                    E2, E3, E4, E5 = pows[1], pows[2], pows[3], pows[4]
                    pairs = [(E, Imask, 1), (E, E, 2), (E2, E, 3), (E2, E2, 4), (E3, E2, 5),
                             (E3, E3, 6), (E4, E3, 7), (E4, E4, 8), (E5, E4, 9), (E5, E5, 10)]
                    ser = None
                    for (Pa, Pb, order_) in pairs:
                        scr2 = S.tile([128, 128], dt.float32, tag="scr2", name="scr2")
                        tr = S.tile([128, 1], dt.float32, tag=f"tr{order_}", name=f"tr{order_}")
                        nc.vector.tensor_tensor(out=scr2[:, :], in0=Pa[:, :], in1=Pb[:, :], op=A_.mult)
                        nc.vector.tensor_reduce(out=tr[:, :], in_=scr2[:, :],
                                                axis=mybir.AxisListType.X, op=A_.add)
                        coef = ((-1.0) ** (order_ + 1)) / order_
                        if ser is None:
                            ser = S.tile([128, 1], dt.float32, tag="ser", name="ser")
                            nc.vector.tensor_scalar(out=ser[:, :], in0=tr[:, :], scalar1=coef,
                                                    scalar2=None, op0=A_.mult)
                        else:
                            ser2 = S.tile([128, 1], dt.float32, tag="ser", name="ser")
                            nc.vector.scalar_tensor_tensor(out=ser2[:, :], in0=tr[:, :],
                                                           scalar=coef, in1=ser[:, :],
                                                           op0=A_.mult, op1=A_.add)
                            ser = ser2
                    tot = S.tile([128, 1], dt.float32, tag="totld", name="totld")
                    nc.vector.scalar_tensor_tensor(out=tot[:, :], in0=lg[:, :], scalar=1.0,
                                                   in1=ser[:, :], op0=A_.mult, op1=A_.add)
                    nxt = 1 - ld_cur
                    nc.vector.scalar_tensor_tensor(out=ld_acc[nxt][:, :], in0=tot[:, :],
                                                   scalar=1.0, in1=ld_acc[ld_cur][:, :],
                                                   op0=A_.mult, op1=A_.add)
                    ld_cur = nxt

            if KSTEPS <= 2:
                st1.__exit__(None, None, None)

            # ---------- finale ----------
            pld = PT.tile([1, 1], dt.float32, tag="pt", name="pt")
            nc.tensor.matmul(pld[:, :], ld_acc[ld_cur][:, :], onesc[:, :], start=True, stop=True)
            ldsum = S.tile([1, 1], dt.float32, tag="ldsum", name="ldsum")
            nc.vector.tensor_copy(out=ldsum[:, :], in_=pld[:, :])
            qd = S.tile([1, 1], dt.float32, tag="qd", name="qd")
            nc.vector.scalar_tensor_tensor(out=qd[:, :], in0=cmid[:, :], scalar=1.0,
                                           in1=cend[:, :], op0=A_.mult, op1=A_.subtract)
            b1 = S.tile([1, 1], dt.float32, tag="b1", name="b1")
            nc.vector.tensor_scalar(out=b1[:, :], in0=qd[:, :], scalar1=0.5,
                                    scalar2=CONST, op0=A_.mult, op1=A_.add)
            lossv = S.tile([1, 1], dt.float32, tag="lossv", name="lossv")
            nc.scalar.activation(out=lossv[:, :], in_=ldsum[:, :], func=AF.Identity,
                                 bias=b1[:, :], scale=0.25)
            nc.sync.dma_start(out=loss_out[:, :], in_=lossv[:, :])

    nc.compile()
    _NC_CACHE = nc
    return nc


LAST_EXEC_NS = None

def kernel(**inputs):
    global LAST_EXEC_NS
    from concourse.bass_utils import run_bass_kernel_spmd
    f32 = np.float32

    def arr(x):
        return np.ascontiguousarray(np.asarray(x, dtype=f32))

    sx, tx = arr(inputs["source_x"]), arr(inputs["target_x"])
    sy, ty = arr(inputs["source_y"]), arr(inputs["target_y"])
    k_ss, k_tt, k_st = arr(inputs["k_ss"]), arr(inputs["k_tt"]), arr(inputs["k_st"])
    Wf, bf = arr(inputs["Wf"]), arr(inputs["bf"])
    Ws, bs = arr(inputs["Ws"]), arr(inputs["bs"])
    Wt, bt = arr(inputs["Wt"]), arr(inputs["bt"])
    Kb = arr(inputs["Kb"])
    base_s, base_t = arr(inputs["base_s"]), arr(inputs["base_t"])
    noise_s, noise_t = arr(inputs["noise_s_opt"]), arr(inputs["noise_t_opt"])

    assert np.all(bf == 0.0), "kernel assumes bf == 0"
    ybcol = np.concatenate([sy[:, 0], ty[:, 0], np.zeros(128, f32)]).reshape(4224, 1).astype(f32)
    ones_row = np.ones((1, 128), f32)
    Imask = np.eye(128, dtype=f32)
    onescol = np.ones((128, 1), f32)
    WfT = np.ascontiguousarray(Wf.T)

    in_maps = []
    for c in range(NCORES):
        # core c owns global column blocks {c, 8+c} (source) and {16+c, 24+c}
        # (target); in index terms: source/target rows [128c:128c+128] and
        # [128(c+8):128(c+8)+128]
        bsel = np.r_[128 * c:128 * c + 128, 128 * (c + 8):128 * (c + 8) + 128]
        nsel = np.zeros((128, 8), f32)
        nsel[:, c] = 1.0
        khad_c = np.empty((4096, 512), f32)
        khad_c[0:2048, 0:256] = k_ss[:, bsel]
        khad_c[0:2048, 256:512] = k_st[:, bsel]
        khad_c[2048:4096, 0:256] = k_st[bsel, :].T
        khad_c[2048:4096, 256:512] = k_tt[:, bsel]
        yrow = np.zeros((1, WSL), f32)
        yrow[0, 0:256] = sy[bsel, 0]
        yrow[0, 256:512] = ty[bsel, 0]
        in_maps.append(dict(
            xsT=np.ascontiguousarray(sx[bsel, :].T),
            xtT=np.ascontiguousarray(tx[bsel, :].T),
            bsT=np.ascontiguousarray(base_s.T),
            btT=np.ascontiguousarray(base_t.T),
            WfT=WfT,
            WaS=np.tile(Ws[0:1, 0:8], (128, 1)).astype(f32),
            WaT=np.tile(Wt[0:1, 0:8], (128, 1)).astype(f32),
            WcS=np.ascontiguousarray(Ws[0, 8:16].reshape(8, 1)),
            WcT=np.ascontiguousarray(Wt[0, 8:16].reshape(8, 1)),
            bS=bs.reshape(1, 1), bT=bt.reshape(1, 1),
            noiseS=noise_s.reshape(1, 1), noiseT=noise_t.reshape(1, 1),
            Kb=Kb, khad=khad_c, ybcol=ybcol, yrow=yrow,
            ones_row=ones_row, Imask=Imask, onescol=onescol,
            nsel=nsel,
        ))

    nc = build()
    trace = bool(int(os.environ.get("KERNEL_TRACE", "0")))
    loss = None
    if os.environ.get("KERNEL_FORCE_SIM", "0") != "1":
        try:
            kw = {}
            td = os.environ.get("KERNEL_TRACE_DIR")
            if td:
                kw["tmpdir"] = td
            res = run_bass_kernel_spmd(nc, in_maps, core_ids=list(range(NCORES)), trace=trace, **kw)
            LAST_EXEC_NS = res.exec_time_ns
            loss = np.float32(res.results[0]["loss"][0, 0])
        except Exception as e:
            sys.stderr.write("HW path failed (%s); falling back to MultiCoreSim\n" % type(e).__name__)
    if loss is None or not np.isfinite(loss):
        from concourse.bass_interp import MultiCoreSim
        sim = MultiCoreSim(nc, num_cores=NCORES, trace=False,
                           require_finite=False, require_nnan=False)
        for i in range(NCORES):
            for kk, vv in in_maps[i].items():
                sim.cores[i].tensor(kk)[:] = vv
        sim.simulate(check_with_hw=False)
        loss = np.float32(sim.cores[0].mem_tensor("loss")[0, 0])
    return np.asarray(loss, dtype=np.float32).reshape(())
